# revision 19
# baseline (speedup 1.0000x reference)
"""Chamfer distance on 8 Trainium2 NeuronCores.

Problem: x1 (8, 4096, 3) f32, y1 (8, 4096, 3) f32.
  d2[b,m,n] = |y[b,m] - x[b,n]|^2
  out = mean_{b,n}(min_m sqrt(1e-8 + max(d2,0))) + mean_{b,m}(min_n ...)

Strategy (data-parallel over B, one batch element per core):
  * sqrt / +eps / max(.,0) are monotonic -> compute mins over raw d2 and
    apply them only to the reduced 4096-vectors on the host.
  * -d2 is produced in PSUM by matmuls with augmented K=30 inputs
    (3-level bf16 split of each fp32 operand, ~2^-26 accurate); the y
    side is negated so all on-device mins become maxes (MAX8 usable).
  * the PE runs TWO row-tiled streams (tile rows 0 and 2 of the 32x128
    tiling grid, operands replicated at SBUF partition bases 0 and 64),
    so the two weight/ifmap streams overlap and LDWEIGHTS hides.
  * per m-tile-PAIR the two streams fill one [128, 2, 4096] bf16 slab
    (ScalarE casts the four 2048-col PSUM chunks; this ~1.9us/chunk
    evacuation is pinned to ScalarE to keep the DVE free).
  * the DVE is the bottleneck (~100% busy): it runs, per pair,
      - direction B (min over m per n): 2 running-max tensor_tensors
        into a [128, 4096] accumulator (bf16 2x mode),
      - direction A (min over n per m): a halving max tree BATCHED over
        the pair via 3-D access patterns (halves per level in one op),
        finishing with MAX8 per tile into m8all.
  * outputs: m8all [128, 32*8] bf16 (host takes max of each 8) and the
    dirB accumulator [128, 4096] bf16 (host takes max over partitions).
    Output DMA is split across queues to shorten the tail.
"""

import os
import sys

for _p in ("/opt/trn_rl_repo", "/root/.axon_site/_ro/trn_rl_repo"):
    if os.path.isdir(_p) and _p not in sys.path:
        sys.path.insert(0, _p)
        break

import numpy as np
import ml_dtypes

_B = 8
_N = 4096          # points per cloud (both x and y)
_K = 30            # augmented contraction dim (3-level bf16 split)
_NCORES = 8
_MT = _N // 128    # 32 m-tiles
_CHUNK = 2048      # PSUM chunk (4 banks); 2 chunks per m-tile

_BF16 = ml_dtypes.bfloat16

# knobs
_STREAMS = int(os.environ.get("CH_STREAMS", "2"))   # 1 or 2 PE tile rows
# "mt:c" chunks cast by the DVE instead of ScalarE (DVE idles in the ramp)
_DVE_CASTS = frozenset(
    tuple(int(v) for v in t.split(":"))
    for t in os.environ.get("CH_DVE_CASTS", "0:0").split(",") if t)

_PROGRAM = None


def _build_program():
    import concourse.bacc as bacc
    import concourse.tile as tile
    import concourse.mybir as mybir

    f32 = mybir.dt.float32
    bf16 = mybir.dt.bfloat16
    MAX = mybir.AluOpType.max

    nc = bacc.Bacc("TRN2", target_bir_lowering=False, debug=False,
                   num_devices=_NCORES)

    xh_d = nc.dram_tensor("xh", [_K, _N], bf16, kind="ExternalInput")
    yh_d = nc.dram_tensor("yh", [_K, _N], bf16, kind="ExternalInput")
    outa_d = nc.dram_tensor("outa", [128, _MT * 8], bf16,
                            kind="ExternalOutput")
    outb_d = nc.dram_tensor("outb", [128, _N], bf16, kind="ExternalOutput")

    with tile.TileContext(nc) as tc:
        with tc.tile_pool(name="singles", bufs=1) as singles:
            # operand replicas at partition bases 0 and 64 (PE tile rows
            # 0 and 2 of the 32x128 row-tiling grid)
            xh_s = singles.tile([128, _N], bf16)
            yh_s = singles.tile([128, _N], bf16)
            m8all = singles.tile([128, _MT * 8], bf16)
            accB = singles.tile([128, _N], bf16)

            # input DMAs: sync + gpsimd queues only (the scalar engine is
            # the saturated caster -- its queue must stay DMA-free).
            # Pieces ordered by when the pipeline needs them.
            bases = (0, 64) if _STREAMS == 2 else (0,)
            qmap = {0: nc.sync, 64: nc.gpsimd}
            for base in bases:
                qa = qmap[base]
                qa.dma_start(out=xh_s[base:base + _K, 0:512],
                             in_=xh_d.ap()[:, 0:512])
                qa.dma_start(out=yh_s[base:base + _K, 0:256],
                             in_=yh_d.ap()[:, 0:256])
                qa.dma_start(out=xh_s[base:base + _K, 512:2048],
                             in_=xh_d.ap()[:, 512:2048])
            for base in bases:
                qa = qmap[base]
                qa.dma_start(out=xh_s[base:base + _K, 2048:_N],
                             in_=xh_d.ap()[:, 2048:_N])
            for base in bases:
                qa = qmap[base]
                qa.dma_start(out=yh_s[base:base + _K, 256:2048],
                             in_=yh_d.ap()[:, 256:2048])
                qa.dma_start(out=yh_s[base:base + _K, 2048:_N],
                             in_=yh_d.ap()[:, 2048:_N])

            with tc.tile_pool(name="psum0", bufs=1, space="PSUM") as psum0, \
                 tc.tile_pool(name="psum1", bufs=1, space="PSUM") as psum1, \
                 tc.tile_pool(name="castp", bufs=4) as castp, \
                 tc.tile_pool(name="treep", bufs=3) as treep:
                psum_pools = (psum0, psum1)
                NPAIR = _MT // 2

                def emit_mms(p, c, r, pt):
                    mt = 2 * p + r
                    rb = 64 * r if _STREAMS == 2 else 0
                    lhsT = yh_s[rb:rb + _K, mt * 128:(mt + 1) * 128]
                    for j in range(_CHUNK // 512):
                        n0 = c * _CHUNK + j * 512
                        nc.tensor.matmul(
                            pt[:, j * 512:(j + 1) * 512],
                            lhsT=lhsT,
                            rhs=xh_s[rb:rb + _K, n0:n0 + 512],
                            start=True, stop=True,
                        )

                def acc_update(mt, lo, hi, src):
                    # running dirB max over tiles for columns [lo:hi)
                    if mt == 0:
                        nc.vector.tensor_copy(accB[:, lo:hi], src)
                    else:
                        nc.vector.tensor_tensor(
                            out=accB[:, lo:hi], in0=accB[:, lo:hi],
                            in1=src, op=MAX)

                def lvl1_piece(h1, r, slab, lo, hi):
                    # dirA level-1 fold of slab columns [lo:hi) -> h1 slot
                    w = (hi - lo) // 2
                    nc.vector.tensor_tensor(
                        out=h1[:, r, lo // 2:lo // 2 + w],
                        in0=slab[:, r, lo:lo + w],
                        in1=slab[:, r, lo + w:hi], op=MAX)

                def dira_upper(p, h1):
                    # tree levels 2..4 batched over the pair + MAX8 per tile
                    h2 = treep.tile([128, 2, 1024], bf16, name="h2")
                    nc.vector.tensor_tensor(
                        out=h2[:, :, :], in0=h1[:, :, 0:1024],
                        in1=h1[:, :, 1024:2048], op=MAX)
                    h3 = treep.tile([128, 2, 512], bf16, name="h3")
                    nc.vector.tensor_tensor(
                        out=h3[:, :, :], in0=h2[:, :, 0:512],
                        in1=h2[:, :, 512:1024], op=MAX)
                    h4 = treep.tile([128, 2, 256], bf16, name="h4")
                    nc.vector.tensor_tensor(
                        out=h4[:, :, :], in0=h3[:, :, 0:256],
                        in1=h3[:, :, 256:512], op=MAX)
                    for r in range(2):
                        mt = 2 * p + r
                        nc.vector.max(m8all[:, mt * 8:(mt + 1) * 8],
                                      h4[:, r, :])

                # ---- ramp: pairs 0-1 run chunk/piece-granular in a
                # c0-wave-then-c1-wave order matching input-DMA arrival,
                # so the DVE starts as soon as the first pieces are cast
                RAMP = 2
                rslabs = [castp.tile([128, 2, _N], bf16, name="slab")
                          for i in range(RAMP)]
                rh1s = [treep.tile([128, 2, _CHUNK], bf16, name="h1")
                        for i in range(RAMP)]
                for c in range(2):
                    for p in range(RAMP):
                        slab, h1 = rslabs[p], rh1s[p]
                        for r in range(2):
                            mt = 2 * p + r
                            pt = psum_pools[r].tile([128, _CHUNK], f32,
                                                    name=f"pt{r}")
                            emit_mms(p, c, r, pt)
                            base = c * _CHUNK
                            if p == 0 and c == 0:
                                # 1024-col cast pieces so the DVE starts
                                # right after the first one lands
                                nc.scalar.copy(
                                    out=slab[:, r, 0:1024],
                                    in_=pt[:, 0:1024])
                                nc.scalar.copy(
                                    out=slab[:, r, 1024:2048],
                                    in_=pt[:, 1024:2048])
                                for lo in (0, 1024):
                                    acc_update(mt, lo, lo + 1024,
                                               slab[:, r, lo:lo + 1024])
                                    lvl1_piece(h1, r, slab, lo, lo + 1024)
                            else:
                                nc.scalar.copy(
                                    out=slab[:, r, base:base + _CHUNK],
                                    in_=pt[:, :])
                                acc_update(mt, base, base + _CHUNK,
                                           slab[:, r, base:base + _CHUNK])
                                lvl1_piece(h1, r, slab, base, base + _CHUNK)
                for p in range(RAMP):
                    dira_upper(p, rh1s[p])

                # ---- steady state: full-tile granularity ----
                for p in range(RAMP, NPAIR - 1):
                    slab = castp.tile([128, 2, _N], bf16, name="slab")
                    for c in range(2):
                        for r in range(2):
                            mt = 2 * p + r
                            pt = psum_pools[r].tile([128, _CHUNK], f32,
                                                    name=f"pt{r}")
                            emit_mms(p, c, r, pt)
                            nc.scalar.copy(
                                out=slab[:, r, c * _CHUNK:(c + 1) * _CHUNK],
                                in_=pt[:, :])
                    for r in range(2):
                        acc_update(2 * p + r, 0, _N, slab[:, r, :])
                    h1 = treep.tile([128, 2, _CHUNK], bf16, name="h1")
                    nc.vector.tensor_tensor(
                        out=h1[:, :, :], in0=slab[:, :, 0:2048],
                        in1=slab[:, :, 2048:4096], op=MAX)
                    dira_upper(p, h1)

                # ---- tail pair: chunk-granular so outb chunk 0 can ship
                # while chunk 1 is still reducing ----
                p = NPAIR - 1
                slab = castp.tile([128, 2, _N], bf16, name="slab")
                h1 = treep.tile([128, 2, _CHUNK], bf16, name="h1")
                for c in range(2):
                    for r in range(2):
                        mt = 2 * p + r
                        pt = psum_pools[r].tile([128, _CHUNK], f32,
                                                name=f"pt{r}")
                        emit_mms(p, c, r, pt)
                        dst = slab[:, r, c * _CHUNK:(c + 1) * _CHUNK]
                        if r == 0:
                            # DVE-assisted cast: by now ScalarE is the
                            # straggler and the DVE would otherwise idle
                            nc.vector.tensor_copy(dst, pt[:, :])
                        else:
                            nc.scalar.copy(out=dst, in_=pt[:, :])
                    base = c * _CHUNK
                    for r in range(2):
                        acc_update(2 * p + r, base, base + _CHUNK,
                                   slab[:, r, base:base + _CHUNK])
                        lvl1_piece(h1, r, slab, base, base + _CHUNK)
                    if c == 0:
                        nc.sync.dma_start(out=outb_d.ap()[:, 0:1024],
                                          in_=accB[:, 0:1024])
                        nc.gpsimd.dma_start(out=outb_d.ap()[:, 1024:2048],
                                            in_=accB[:, 1024:2048])
                dira_upper(p, h1)

            # ---- epilogue DMA (split across queues to hide the tail) ----
            nc.gpsimd.dma_start(out=outb_d.ap()[:, 2048:3072],
                                in_=accB[:, 2048:3072])
            nc.sync.dma_start(out=outb_d.ap()[:, 3072:_N],
                              in_=accB[:, 3072:_N])
            nc.sync.dma_start(out=outa_d.ap()[:, 0:128],
                              in_=m8all[:, 0:128])
            nc.gpsimd.dma_start(out=outa_d.ap()[:, 128:256],
                                in_=m8all[:, 128:256])

    nc.compile()
    return nc


def _get_program():
    global _PROGRAM
    if _PROGRAM is None:
        _PROGRAM = _build_program()
    return _PROGRAM


def _split3(a):
    """fp32 array -> 3-level bf16 split (h1 + h2 + h3 ~ a to ~2^-26 rel)."""
    h1 = a.astype(_BF16)
    r1 = a - h1.astype(np.float32)
    h2 = r1.astype(_BF16)
    r2 = r1 - h2.astype(np.float32)
    h3 = r2.astype(_BF16)
    return h1, h2, h3


def _augment(x, y):
    """x, y: (4096, 3) f32 -> xh, yh (30, 4096) bf16 such that
    sum_k yh[k, m] * xh[k, n] == -|y[m] - x[n]|^2 to ~1e-6 abs.

    Every fp32 operand is split into 3 bf16 levels; all product pairs down
    to the 2^-24 level are kept, so each product is exact in the PE's fp32
    PSUM accumulation.  Large-magnitude rows (y_sq, x_sq, hi*hi cross
    terms) come first so the running PSUM partial cancels down to ~d2
    early, keeping sequential-accumulation rounding at the fp32 noise
    floor of the reference itself."""
    xt = np.ascontiguousarray(x.T.astype(np.float32))            # (3, N)
    y2t = np.ascontiguousarray((-2.0 * y).T.astype(np.float32))  # (3, N)
    xsq = np.einsum("nd,nd->n", x, x).astype(np.float32)         # (N,)
    ysq = np.einsum("nd,nd->n", y, y).astype(np.float32)

    g1, g2, g3 = _split3(xt)
    h1, h2, h3 = _split3(y2t)
    xs1, xs2, xs3 = _split3(xsq)
    ys1, ys2, ys3 = _split3(ysq)
    ones = np.ones(_N, dtype=_BF16)

    xrows, yrows = [], []

    def add(xr, yr):
        xrows.append(xr)
        yrows.append(yr)

    add(ones, ys1)
    add(xs1, ones)
    for d in range(3):
        add(g1[d], h1[d])
    add(ones, ys2)
    add(ones, ys3)
    add(xs2, ones)
    add(xs3, ones)
    for d in range(3):
        add(g2[d], h1[d])
        add(g1[d], h2[d])
        add(g3[d], h1[d])
        add(g2[d], h2[d])
        add(g1[d], h3[d])
        add(g3[d], h2[d])
        add(g2[d], h3[d])
    xh = np.stack(xrows).astype(_BF16)
    # negate the y side so the PE emits -d2 (mins become maxes on-device)
    yh = (-np.stack(yrows).astype(np.float32)).astype(_BF16)
    assert xh.shape == (_K, _N)
    return xh, yh


def kernel(x1, y1):
    from concourse.bass_utils import run_bass_kernel_spmd

    x1 = np.asarray(x1)
    y1 = np.asarray(y1)
    assert x1.shape == (_B, _N, 3) and y1.shape == (_B, _N, 3)

    nc = _get_program()
    in_maps = []
    for b in range(_B):
        xh, yh = _augment(x1[b], y1[b])
        in_maps.append({"xh": xh, "yh": yh})

    res = run_bass_kernel_spmd(nc, in_maps, list(range(_NCORES)))
    total = 0.0
    for c in range(_NCORES):
        ma = res.results[c]["outa"].astype(np.float32)  # (128, 32*8)
        mb = res.results[c]["outb"].astype(np.float32)  # (128, 4096)
        a = ma.reshape(128, _MT, 8).max(axis=2)         # -d2min per m
        b = mb.max(axis=0)                              # -d2min per n
        dist_a = np.sqrt(1.0e-8 + np.maximum(-a, 0.0), dtype=np.float32)
        dist_b = np.sqrt(1.0e-8 + np.maximum(-b, 0.0), dtype=np.float32)
        total += float(dist_a.sum(dtype=np.float64))
        total += float(dist_b.sum(dtype=np.float64))
    return np.float32(total / (_B * _N))


# revision 20
# speedup vs baseline: 1.0219x; 1.0219x over previous
"""Chamfer distance on 8 Trainium2 NeuronCores.

Problem: x1 (8, 4096, 3) f32, y1 (8, 4096, 3) f32.
  d2[b,m,n] = |y[b,m] - x[b,n]|^2
  out = mean_{b,n}(min_m sqrt(1e-8 + max(d2,0))) + mean_{b,m}(min_n ...)

Strategy (data-parallel over B, one batch element per core):
  * sqrt / +eps / max(.,0) are monotonic -> compute mins over raw d2 and
    apply them only to the reduced 4096-vectors on the host.
  * -d2 is produced in PSUM by matmuls with augmented K=30 inputs
    (3-level bf16 split of each fp32 operand, ~2^-26 accurate); the y
    side is negated so all on-device mins become maxes (MAX8 usable).
  * the PE runs TWO row-tiled streams (tile rows 0 and 2 of the 32x128
    tiling grid, operands replicated at SBUF partition bases 0 and 64),
    so the two weight/ifmap streams overlap and LDWEIGHTS hides.
  * per m-tile-PAIR the two streams fill one [128, 2, 4096] bf16 slab
    (ScalarE casts the four 2048-col PSUM chunks; this ~1.9us/chunk
    evacuation is pinned to ScalarE to keep the DVE free).
  * the DVE is the bottleneck (~100% busy): it runs, per pair,
      - direction B (min over m per n): 2 running-max tensor_tensors
        into a [128, 4096] accumulator (bf16 2x mode),
      - direction A (min over n per m): a halving max tree BATCHED over
        the pair via 3-D access patterns (halves per level in one op),
        finishing with MAX8 per tile into m8all.
  * outputs: m8all [128, 32*8] bf16 (host takes max of each 8) and the
    dirB accumulator [128, 4096] bf16 (host takes max over partitions).
    Output DMA is split across queues to shorten the tail.
"""

import os
import sys

for _p in ("/opt/trn_rl_repo", "/root/.axon_site/_ro/trn_rl_repo"):
    if os.path.isdir(_p) and _p not in sys.path:
        sys.path.insert(0, _p)
        break

import numpy as np
import ml_dtypes

_B = 8
_N = 4096          # points per cloud (both x and y)
_K = 30            # augmented contraction dim (3-level bf16 split)
_NCORES = 8
_MT = _N // 128    # 32 m-tiles
_CHUNK = 2048      # PSUM chunk (4 banks); 2 chunks per m-tile

_BF16 = ml_dtypes.bfloat16

# knobs
_STREAMS = int(os.environ.get("CH_STREAMS", "2"))   # 1 or 2 PE tile rows
# "mt:c" chunks cast by the DVE instead of ScalarE (DVE idles in the ramp)
_DVE_CASTS = frozenset(
    tuple(int(v) for v in t.split(":"))
    for t in os.environ.get("CH_DVE_CASTS", "0:0").split(",") if t)

_PROGRAM = None


def _build_program():
    import concourse.bacc as bacc
    import concourse.tile as tile
    import concourse.mybir as mybir

    f32 = mybir.dt.float32
    bf16 = mybir.dt.bfloat16
    MAX = mybir.AluOpType.max

    nc = bacc.Bacc("TRN2", target_bir_lowering=False, debug=False,
                   num_devices=_NCORES)

    xh_d = nc.dram_tensor("xh", [_K, _N], bf16, kind="ExternalInput")
    yh_d = nc.dram_tensor("yh", [_K, _N], bf16, kind="ExternalInput")
    outa_d = nc.dram_tensor("outa", [128, _MT * 8], bf16,
                            kind="ExternalOutput")
    outb_d = nc.dram_tensor("outb", [128, _N], bf16, kind="ExternalOutput")

    with tile.TileContext(nc) as tc:
        with tc.tile_pool(name="singles", bufs=1) as singles:
            # operand replicas at partition bases 0 and 64 (PE tile rows
            # 0 and 2 of the 32x128 row-tiling grid)
            xh_s = singles.tile([128, _N], bf16)
            yh_s = singles.tile([128, _N], bf16)
            m8all = singles.tile([128, _MT * 8], bf16)
            accB = singles.tile([128, _N], bf16)

            # input DMAs: sync + gpsimd queues only (the scalar engine is
            # the saturated caster -- its queue must stay DMA-free).
            # Pieces ordered by when the pipeline needs them.
            bases = (0, 64) if _STREAMS == 2 else (0,)
            qmap = {0: nc.sync, 64: nc.gpsimd}
            for base in bases:
                qa = qmap[base]
                qa.dma_start(out=xh_s[base:base + _K, 0:512],
                             in_=xh_d.ap()[:, 0:512])
                qa.dma_start(out=yh_s[base:base + _K, 0:256],
                             in_=yh_d.ap()[:, 0:256])
                qa.dma_start(out=xh_s[base:base + _K, 512:2048],
                             in_=xh_d.ap()[:, 512:2048])
            for base in bases:
                qa = qmap[base]
                qa.dma_start(out=xh_s[base:base + _K, 2048:_N],
                             in_=xh_d.ap()[:, 2048:_N])
            for base in bases:
                qa = qmap[base]
                qa.dma_start(out=yh_s[base:base + _K, 256:2048],
                             in_=yh_d.ap()[:, 256:2048])
                qa.dma_start(out=yh_s[base:base + _K, 2048:_N],
                             in_=yh_d.ap()[:, 2048:_N])

            with tc.tile_pool(name="psum0", bufs=1, space="PSUM") as psum0, \
                 tc.tile_pool(name="psum1", bufs=1, space="PSUM") as psum1, \
                 tc.tile_pool(name="castp", bufs=4) as castp, \
                 tc.tile_pool(name="treep", bufs=3) as treep:
                psum_pools = (psum0, psum1)
                NPAIR = _MT // 2

                def emit_mms(p, c, r, pt):
                    mt = 2 * p + r
                    rb = 64 * r if _STREAMS == 2 else 0
                    lhsT = yh_s[rb:rb + _K, mt * 128:(mt + 1) * 128]
                    for j in range(_CHUNK // 512):
                        n0 = c * _CHUNK + j * 512
                        nc.tensor.matmul(
                            pt[:, j * 512:(j + 1) * 512],
                            lhsT=lhsT,
                            rhs=xh_s[rb:rb + _K, n0:n0 + 512],
                            start=True, stop=True,
                        )

                def acc_update(mt, lo, hi, src):
                    # running dirB max over tiles for columns [lo:hi)
                    if mt == 0:
                        nc.vector.tensor_copy(accB[:, lo:hi], src)
                    else:
                        nc.vector.tensor_tensor(
                            out=accB[:, lo:hi], in0=accB[:, lo:hi],
                            in1=src, op=MAX)

                def lvl1_piece(h1, r, slab, lo, hi):
                    # dirA level-1 fold of slab columns [lo:hi) -> h1 slot
                    w = (hi - lo) // 2
                    nc.vector.tensor_tensor(
                        out=h1[:, r, lo // 2:lo // 2 + w],
                        in0=slab[:, r, lo:lo + w],
                        in1=slab[:, r, lo + w:hi], op=MAX)

                def dira_upper(p, h1):
                    # tree levels 2..4 batched over the pair + MAX8 per tile
                    h2 = treep.tile([128, 2, 1024], bf16, name="h2")
                    nc.vector.tensor_tensor(
                        out=h2[:, :, :], in0=h1[:, :, 0:1024],
                        in1=h1[:, :, 1024:2048], op=MAX)
                    h3 = treep.tile([128, 2, 512], bf16, name="h3")
                    nc.vector.tensor_tensor(
                        out=h3[:, :, :], in0=h2[:, :, 0:512],
                        in1=h2[:, :, 512:1024], op=MAX)
                    h4 = treep.tile([128, 2, 256], bf16, name="h4")
                    nc.vector.tensor_tensor(
                        out=h4[:, :, :], in0=h3[:, :, 0:256],
                        in1=h3[:, :, 256:512], op=MAX)
                    for r in range(2):
                        mt = 2 * p + r
                        nc.vector.max(m8all[:, mt * 8:(mt + 1) * 8],
                                      h4[:, r, :])

                # ---- ramp: pairs 0-1 run chunk/piece-granular in a
                # c0-wave-then-c1-wave order matching input-DMA arrival,
                # so the DVE starts as soon as the first pieces are cast
                RAMP = 2
                rslabs = [castp.tile([128, 2, _N], bf16, name="slab")
                          for i in range(RAMP)]
                rh1s = [treep.tile([128, 2, _CHUNK], bf16, name="h1")
                        for i in range(RAMP)]
                for c in range(2):
                    for p in range(RAMP):
                        slab, h1 = rslabs[p], rh1s[p]
                        for r in range(2):
                            mt = 2 * p + r
                            pt = psum_pools[r].tile([128, _CHUNK], f32,
                                                    name=f"pt{r}")
                            emit_mms(p, c, r, pt)
                            base = c * _CHUNK
                            if p == 0 and c == 0:
                                # 1024-col cast pieces so the DVE starts
                                # right after the first one lands
                                nc.scalar.copy(
                                    out=slab[:, r, 0:1024],
                                    in_=pt[:, 0:1024])
                                nc.scalar.copy(
                                    out=slab[:, r, 1024:2048],
                                    in_=pt[:, 1024:2048])
                                for lo in (0, 1024):
                                    acc_update(mt, lo, lo + 1024,
                                               slab[:, r, lo:lo + 1024])
                                    lvl1_piece(h1, r, slab, lo, lo + 1024)
                            else:
                                nc.scalar.copy(
                                    out=slab[:, r, base:base + _CHUNK],
                                    in_=pt[:, :])
                                acc_update(mt, base, base + _CHUNK,
                                           slab[:, r, base:base + _CHUNK])
                                lvl1_piece(h1, r, slab, base, base + _CHUNK)
                for p in range(RAMP):
                    dira_upper(p, rh1s[p])

                # ---- steady state: full-tile granularity ----
                for p in range(RAMP, NPAIR - 1):
                    slab = castp.tile([128, 2, _N], bf16, name="slab")
                    for c in range(2):
                        for r in range(2):
                            mt = 2 * p + r
                            pt = psum_pools[r].tile([128, _CHUNK], f32,
                                                    name=f"pt{r}")
                            emit_mms(p, c, r, pt)
                            nc.scalar.copy(
                                out=slab[:, r, c * _CHUNK:(c + 1) * _CHUNK],
                                in_=pt[:, :])
                    for r in range(2):
                        acc_update(2 * p + r, 0, _N, slab[:, r, :])
                    h1 = treep.tile([128, 2, _CHUNK], bf16, name="h1")
                    nc.vector.tensor_tensor(
                        out=h1[:, :, :], in0=slab[:, :, 0:2048],
                        in1=slab[:, :, 2048:4096], op=MAX)
                    dira_upper(p, h1)

                # ---- tail pair: chunk-granular so outb chunk 0 can ship
                # while chunk 1 is still reducing ----
                p = NPAIR - 1
                slab = castp.tile([128, 2, _N], bf16, name="slab")
                h1 = treep.tile([128, 2, _CHUNK], bf16, name="h1")
                for c in range(2):
                    for r in range(2):
                        mt = 2 * p + r
                        pt = psum_pools[r].tile([128, _CHUNK], f32,
                                                name=f"pt{r}")
                        emit_mms(p, c, r, pt)
                        nc.scalar.copy(
                            out=slab[:, r, c * _CHUNK:(c + 1) * _CHUNK],
                            in_=pt[:, :])
                    base = c * _CHUNK
                    for r in range(2):
                        acc_update(2 * p + r, base, base + _CHUNK,
                                   slab[:, r, base:base + _CHUNK])
                        lvl1_piece(h1, r, slab, base, base + _CHUNK)
                    if c == 0:
                        nc.sync.dma_start(out=outb_d.ap()[:, 0:1024],
                                          in_=accB[:, 0:1024])
                        nc.gpsimd.dma_start(out=outb_d.ap()[:, 1024:2048],
                                            in_=accB[:, 1024:2048])
                dira_upper(p, h1)

            # ---- epilogue DMA (split across queues to hide the tail) ----
            nc.gpsimd.dma_start(out=outb_d.ap()[:, 2048:3072],
                                in_=accB[:, 2048:3072])
            nc.sync.dma_start(out=outb_d.ap()[:, 3072:_N],
                              in_=accB[:, 3072:_N])
            nc.sync.dma_start(out=outa_d.ap()[:, 0:128],
                              in_=m8all[:, 0:128])
            nc.gpsimd.dma_start(out=outa_d.ap()[:, 128:256],
                                in_=m8all[:, 128:256])

    nc.compile()
    return nc


def _get_program():
    global _PROGRAM
    if _PROGRAM is None:
        _PROGRAM = _build_program()
    return _PROGRAM


def _split3(a):
    """fp32 array -> 3-level bf16 split (h1 + h2 + h3 ~ a to ~2^-26 rel)."""
    h1 = a.astype(_BF16)
    r1 = a - h1.astype(np.float32)
    h2 = r1.astype(_BF16)
    r2 = r1 - h2.astype(np.float32)
    h3 = r2.astype(_BF16)
    return h1, h2, h3


def _augment(x, y):
    """x, y: (4096, 3) f32 -> xh, yh (30, 4096) bf16 such that
    sum_k yh[k, m] * xh[k, n] == -|y[m] - x[n]|^2 to ~1e-6 abs.

    Every fp32 operand is split into 3 bf16 levels; all product pairs down
    to the 2^-24 level are kept, so each product is exact in the PE's fp32
    PSUM accumulation.  Large-magnitude rows (y_sq, x_sq, hi*hi cross
    terms) come first so the running PSUM partial cancels down to ~d2
    early, keeping sequential-accumulation rounding at the fp32 noise
    floor of the reference itself."""
    xt = np.ascontiguousarray(x.T.astype(np.float32))            # (3, N)
    y2t = np.ascontiguousarray((-2.0 * y).T.astype(np.float32))  # (3, N)
    xsq = np.einsum("nd,nd->n", x, x).astype(np.float32)         # (N,)
    ysq = np.einsum("nd,nd->n", y, y).astype(np.float32)

    g1, g2, g3 = _split3(xt)
    h1, h2, h3 = _split3(y2t)
    xs1, xs2, xs3 = _split3(xsq)
    ys1, ys2, ys3 = _split3(ysq)
    ones = np.ones(_N, dtype=_BF16)

    xrows, yrows = [], []

    def add(xr, yr):
        xrows.append(xr)
        yrows.append(yr)

    add(ones, ys1)
    add(xs1, ones)
    for d in range(3):
        add(g1[d], h1[d])
    add(ones, ys2)
    add(ones, ys3)
    add(xs2, ones)
    add(xs3, ones)
    for d in range(3):
        add(g2[d], h1[d])
        add(g1[d], h2[d])
        add(g3[d], h1[d])
        add(g2[d], h2[d])
        add(g1[d], h3[d])
        add(g3[d], h2[d])
        add(g2[d], h3[d])
    xh = np.stack(xrows).astype(_BF16)
    # negate the y side so the PE emits -d2 (mins become maxes on-device)
    yh = (-np.stack(yrows).astype(np.float32)).astype(_BF16)
    assert xh.shape == (_K, _N)
    return xh, yh


def kernel(x1, y1):
    from concourse.bass_utils import run_bass_kernel_spmd

    x1 = np.asarray(x1)
    y1 = np.asarray(y1)
    assert x1.shape == (_B, _N, 3) and y1.shape == (_B, _N, 3)

    nc = _get_program()
    in_maps = []
    for b in range(_B):
        xh, yh = _augment(x1[b], y1[b])
        in_maps.append({"xh": xh, "yh": yh})

    res = run_bass_kernel_spmd(nc, in_maps, list(range(_NCORES)))
    total = 0.0
    for c in range(_NCORES):
        ma = res.results[c]["outa"].astype(np.float32)  # (128, 32*8)
        mb = res.results[c]["outb"].astype(np.float32)  # (128, 4096)
        a = ma.reshape(128, _MT, 8).max(axis=2)         # -d2min per m
        b = mb.max(axis=0)                              # -d2min per n
        dist_a = np.sqrt(1.0e-8 + np.maximum(-a, 0.0), dtype=np.float32)
        dist_b = np.sqrt(1.0e-8 + np.maximum(-b, 0.0), dtype=np.float32)
        total += float(dist_a.sum(dtype=np.float64))
        total += float(dist_b.sum(dtype=np.float64))
    return np.float32(total / (_B * _N))


# revision 25
# speedup vs baseline: 1.1208x; 1.0968x over previous
"""Chamfer distance on 8 Trainium2 NeuronCores.

Problem: x1 (8, 4096, 3) f32, y1 (8, 4096, 3) f32.
  d2[b,m,n] = |y[b,m] - x[b,n]|^2
  out = mean_{b,n}(min_m sqrt(1e-8 + max(d2,0))) + mean_{b,m}(min_n ...)

Strategy (data-parallel over B, one batch element per core):
  * sqrt / +eps / max(.,0) are monotonic -> compute mins over raw d2 and
    apply them only to the reduced 4096-vectors on the host.
  * -d2 is produced in PSUM by matmuls with augmented K=30 inputs
    (3-level bf16 split of each fp32 operand, ~2^-26 accurate); the y
    side is negated so all on-device mins become maxes (MAX8 usable).
  * the PE runs TWO row-tiled streams (tile rows 0 and 2 of the 32x128
    tiling grid, operands replicated at SBUF partition bases 0 and 64),
    so the two weight/ifmap streams overlap and LDWEIGHTS hides.
  * per m-tile-PAIR the two streams fill one [128, 2, 4096] bf16 slab
    (ScalarE casts the four 2048-col PSUM chunks; this ~1.9us/chunk
    evacuation is pinned to ScalarE to keep the DVE free).
  * the DVE is the bottleneck (~100% busy): it runs, per pair,
      - direction B (min over m per n): 2 running-max tensor_tensors
        into a [128, 4096] accumulator (bf16 2x mode),
      - direction A (min over n per m): a halving max tree BATCHED over
        the pair via 3-D access patterns (halves per level in one op),
        finishing with MAX8 per tile into m8all.
  * outputs: m8all [128, 32*8] bf16 (host takes max of each 8) and the
    dirB accumulator [128, 4096] bf16 (host takes max over partitions).
    Output DMA is split across queues to shorten the tail.
"""

import os
import sys

for _p in ("/opt/trn_rl_repo", "/root/.axon_site/_ro/trn_rl_repo"):
    if os.path.isdir(_p) and _p not in sys.path:
        sys.path.insert(0, _p)
        break

import numpy as np
import ml_dtypes

_B = 8
_N = 4096          # points per cloud (both x and y)
_K = 30            # augmented contraction dim (3-level bf16 split)
_NCORES = 8
_MT = _N // 128    # 32 m-tiles
_CHUNK = 2048      # PSUM chunk (4 banks); 2 chunks per m-tile

_BF16 = ml_dtypes.bfloat16

# knobs
_STREAMS = int(os.environ.get("CH_STREAMS", "2"))   # 1 or 2 PE tile rows
# early tiles excluded from the on-device dirB chain; their bf16 slabs
# ship to DRAM (overlapped DMA) and the host folds them in
_SKIP_TILES = tuple(int(t) for t in
                    os.environ.get("CH_SKIP_TILES", "2,5").split(",") if t)

_PROGRAM = None


def _build_program():
    import concourse.bacc as bacc
    import concourse.tile as tile
    import concourse.mybir as mybir

    f32 = mybir.dt.float32
    bf16 = mybir.dt.bfloat16
    MAX = mybir.AluOpType.max

    nc = bacc.Bacc("TRN2", target_bir_lowering=False, debug=False,
                   num_devices=_NCORES)

    xh_d = nc.dram_tensor("xh", [_K, _N], bf16, kind="ExternalInput")
    yh_d = nc.dram_tensor("yh", [_K, _N], bf16, kind="ExternalInput")
    # dirA tree shipped at the h3 level (512 cols per tile) for pairs
    # 0..14; the final pair finishes on-device (avoids a tail DMA)
    outh_d = nc.dram_tensor("outh", [128, 15 * 1024], bf16,
                            kind="ExternalOutput")
    outa_d = nc.dram_tensor("outa", [128, 16], bf16, kind="ExternalOutput")
    outb_d = nc.dram_tensor("outb", [128, _N], bf16, kind="ExternalOutput")
    outs_d = [nc.dram_tensor(f"outs{i}", [128, _N], bf16,
                             kind="ExternalOutput")
              for i in range(len(_SKIP_TILES))]

    with tile.TileContext(nc) as tc:
        with tc.tile_pool(name="singles", bufs=1) as singles:
            # operand replicas at partition bases 0 and 64 (PE tile rows
            # 0 and 2 of the 32x128 row-tiling grid)
            xh_s = singles.tile([128, _N], bf16)
            yh_s = singles.tile([128, _N], bf16)
            m8all = singles.tile([128, 16], bf16)
            accB = singles.tile([128, _N], bf16)

            # input DMAs: sync + gpsimd queues only (the scalar engine is
            # the saturated caster -- its queue must stay DMA-free).
            # Pieces ordered by when the pipeline needs them.
            bases = (0, 64) if _STREAMS == 2 else (0,)
            qmap = {0: nc.sync, 64: nc.gpsimd}
            for base in bases:
                qa = qmap[base]
                qa.dma_start(out=xh_s[base:base + _K, 0:512],
                             in_=xh_d.ap()[:, 0:512])
                qa.dma_start(out=yh_s[base:base + _K, 0:256],
                             in_=yh_d.ap()[:, 0:256])
                qa.dma_start(out=xh_s[base:base + _K, 512:2048],
                             in_=xh_d.ap()[:, 512:2048])
            for base in bases:
                qa = qmap[base]
                qa.dma_start(out=xh_s[base:base + _K, 2048:_N],
                             in_=xh_d.ap()[:, 2048:_N])
            for base in bases:
                qa = qmap[base]
                qa.dma_start(out=yh_s[base:base + _K, 256:2048],
                             in_=yh_d.ap()[:, 256:2048])
                qa.dma_start(out=yh_s[base:base + _K, 2048:_N],
                             in_=yh_d.ap()[:, 2048:_N])

            with tc.tile_pool(name="psum0", bufs=1, space="PSUM") as psum0, \
                 tc.tile_pool(name="psum1", bufs=1, space="PSUM") as psum1, \
                 tc.tile_pool(name="castp", bufs=4) as castp, \
                 tc.tile_pool(name="treep", bufs=3) as treep, \
                 tc.tile_pool(name="h3p", bufs=4) as h3p:
                psum_pools = (psum0, psum1)
                NPAIR = _MT // 2

                def emit_mms(p, c, r, pt):
                    mt = 2 * p + r
                    rb = 64 * r if _STREAMS == 2 else 0
                    lhsT = yh_s[rb:rb + _K, mt * 128:(mt + 1) * 128]
                    for j in range(_CHUNK // 512):
                        n0 = c * _CHUNK + j * 512
                        nc.tensor.matmul(
                            pt[:, j * 512:(j + 1) * 512],
                            lhsT=lhsT,
                            rhs=xh_s[rb:rb + _K, n0:n0 + 512],
                            start=True, stop=True,
                        )

                def acc_update(mt, lo, hi, src):
                    # running dirB max over tiles for columns [lo:hi)
                    if mt == 0:
                        nc.vector.tensor_copy(accB[:, lo:hi], src)
                    else:
                        nc.vector.tensor_tensor(
                            out=accB[:, lo:hi], in0=accB[:, lo:hi],
                            in1=src, op=MAX)

                def lvl1_piece(h1, r, slab, lo, hi):
                    # dirA level-1 fold of slab columns [lo:hi) -> h1 slot
                    w = (hi - lo) // 2
                    nc.vector.tensor_tensor(
                        out=h1[:, r, lo // 2:lo // 2 + w],
                        in0=slab[:, r, lo:lo + w],
                        in1=slab[:, r, lo + w:hi], op=MAX)

                def dira_upper(p, h1):
                    # tree levels 2..3 batched over the pair; the h3
                    # remnant ships to the host (overlapped DMA) except
                    # for the final pair, which finishes on-device
                    h2 = treep.tile([128, 2, 1024], bf16, name="h2")
                    nc.vector.tensor_tensor(
                        out=h2[:, :, :], in0=h1[:, :, 0:1024],
                        in1=h1[:, :, 1024:2048], op=MAX)
                    h3 = h3p.tile([128, 2, 512], bf16, name="h3")
                    nc.vector.tensor_tensor(
                        out=h3[:, :, :], in0=h2[:, :, 0:512],
                        in1=h2[:, :, 512:1024], op=MAX)
                    if p == NPAIR - 1:
                        h4 = treep.tile([128, 2, 256], bf16, name="h4")
                        nc.vector.tensor_tensor(
                            out=h4[:, :, :], in0=h3[:, :, 0:256],
                            in1=h3[:, :, 256:512], op=MAX)
                        for r in range(2):
                            nc.vector.max(m8all[:, r * 8:(r + 1) * 8],
                                          h4[:, r, :])
                    else:
                        q = nc.sync if p % 2 == 0 else nc.gpsimd
                        q.dma_start(
                            out=outh_d.ap()[:, p * 1024:(p + 1) * 1024],
                            in_=h3[:, :, :])

                # ---- ramp: pairs 0-1 run chunk/piece-granular in a
                # c0-wave-then-c1-wave order matching input-DMA arrival,
                # so the DVE starts as soon as the first pieces are cast
                RAMP = 2
                rslabs = [castp.tile([128, 2, _N], bf16, name="slab")
                          for i in range(RAMP)]
                rh1s = [treep.tile([128, 2, _CHUNK], bf16, name="h1")
                        for i in range(RAMP)]
                for c in range(2):
                    for p in range(RAMP):
                        slab, h1 = rslabs[p], rh1s[p]
                        for r in range(2):
                            mt = 2 * p + r
                            pt = psum_pools[r].tile([128, _CHUNK], f32,
                                                    name=f"pt{r}")
                            emit_mms(p, c, r, pt)
                            base = c * _CHUNK
                            if p == 0 and c == 0:
                                # 1024-col cast pieces so the DVE starts
                                # right after the first one lands
                                nc.scalar.copy(
                                    out=slab[:, r, 0:1024],
                                    in_=pt[:, 0:1024])
                                nc.scalar.copy(
                                    out=slab[:, r, 1024:2048],
                                    in_=pt[:, 1024:2048])
                                for lo in (0, 1024):
                                    if mt not in _SKIP_TILES:
                                        acc_update(mt, lo, lo + 1024,
                                                   slab[:, r, lo:lo + 1024])
                                    lvl1_piece(h1, r, slab, lo, lo + 1024)
                            else:
                                nc.scalar.copy(
                                    out=slab[:, r, base:base + _CHUNK],
                                    in_=pt[:, :])
                                if mt not in _SKIP_TILES:
                                    acc_update(mt, base, base + _CHUNK,
                                               slab[:, r,
                                                    base:base + _CHUNK])
                                lvl1_piece(h1, r, slab, base, base + _CHUNK)
                skip_idx = {mt: i for i, mt in enumerate(_SKIP_TILES)}
                for p in range(RAMP):
                    for r in range(2):
                        mt = 2 * p + r
                        if mt in _SKIP_TILES:
                            q = nc.sync if skip_idx[mt] % 2 else nc.gpsimd
                            q.dma_start(out=outs_d[skip_idx[mt]].ap(),
                                        in_=rslabs[p][:, r, :])
                for p in range(RAMP):
                    dira_upper(p, rh1s[p])

                # ---- steady state: full-tile granularity ----
                for p in range(RAMP, NPAIR - 1):
                    slab = castp.tile([128, 2, _N], bf16, name="slab")
                    for c in range(2):
                        for r in range(2):
                            mt = 2 * p + r
                            pt = psum_pools[r].tile([128, _CHUNK], f32,
                                                    name=f"pt{r}")
                            emit_mms(p, c, r, pt)
                            nc.scalar.copy(
                                out=slab[:, r, c * _CHUNK:(c + 1) * _CHUNK],
                                in_=pt[:, :])
                    for r in range(2):
                        mt = 2 * p + r
                        if mt in _SKIP_TILES:
                            q = nc.sync if skip_idx[mt] % 2 else nc.gpsimd
                            q.dma_start(out=outs_d[skip_idx[mt]].ap(),
                                        in_=slab[:, r, :])
                        else:
                            acc_update(mt, 0, _N, slab[:, r, :])
                    h1 = treep.tile([128, 2, _CHUNK], bf16, name="h1")
                    nc.vector.tensor_tensor(
                        out=h1[:, :, :], in0=slab[:, :, 0:2048],
                        in1=slab[:, :, 2048:4096], op=MAX)
                    dira_upper(p, h1)

                # ---- tail pair: chunk-granular so outb chunk 0 can ship
                # while chunk 1 is still reducing ----
                p = NPAIR - 1
                slab = castp.tile([128, 2, _N], bf16, name="slab")
                h1 = treep.tile([128, 2, _CHUNK], bf16, name="h1")
                for c in range(2):
                    for r in range(2):
                        mt = 2 * p + r
                        pt = psum_pools[r].tile([128, _CHUNK], f32,
                                                name=f"pt{r}")
                        emit_mms(p, c, r, pt)
                        nc.scalar.copy(
                            out=slab[:, r, c * _CHUNK:(c + 1) * _CHUNK],
                            in_=pt[:, :])
                    base = c * _CHUNK
                    for r in range(2):
                        acc_update(2 * p + r, base, base + _CHUNK,
                                   slab[:, r, base:base + _CHUNK])
                        lvl1_piece(h1, r, slab, base, base + _CHUNK)
                    if c == 0:
                        nc.sync.dma_start(out=outb_d.ap()[:, 0:1024],
                                          in_=accB[:, 0:1024])
                        nc.gpsimd.dma_start(out=outb_d.ap()[:, 1024:2048],
                                            in_=accB[:, 1024:2048])
                dira_upper(p, h1)

            # ---- epilogue DMA (split across queues to hide the tail) ----
            nc.gpsimd.dma_start(out=outb_d.ap()[:, 2048:3072],
                                in_=accB[:, 2048:3072])
            nc.sync.dma_start(out=outb_d.ap()[:, 3072:_N],
                              in_=accB[:, 3072:_N])
            nc.sync.dma_start(out=outa_d.ap(), in_=m8all[:, :])

    nc.compile()
    return nc


def _get_program():
    global _PROGRAM
    if _PROGRAM is None:
        _PROGRAM = _build_program()
    return _PROGRAM


def _split3(a):
    """fp32 array -> 3-level bf16 split (h1 + h2 + h3 ~ a to ~2^-26 rel)."""
    h1 = a.astype(_BF16)
    r1 = a - h1.astype(np.float32)
    h2 = r1.astype(_BF16)
    r2 = r1 - h2.astype(np.float32)
    h3 = r2.astype(_BF16)
    return h1, h2, h3


def _augment(x, y):
    """x, y: (4096, 3) f32 -> xh, yh (30, 4096) bf16 such that
    sum_k yh[k, m] * xh[k, n] == -|y[m] - x[n]|^2 to ~1e-6 abs.

    Every fp32 operand is split into 3 bf16 levels; all product pairs down
    to the 2^-24 level are kept, so each product is exact in the PE's fp32
    PSUM accumulation.  Large-magnitude rows (y_sq, x_sq, hi*hi cross
    terms) come first so the running PSUM partial cancels down to ~d2
    early, keeping sequential-accumulation rounding at the fp32 noise
    floor of the reference itself."""
    xt = np.ascontiguousarray(x.T.astype(np.float32))            # (3, N)
    y2t = np.ascontiguousarray((-2.0 * y).T.astype(np.float32))  # (3, N)
    xsq = np.einsum("nd,nd->n", x, x).astype(np.float32)         # (N,)
    ysq = np.einsum("nd,nd->n", y, y).astype(np.float32)

    g1, g2, g3 = _split3(xt)
    h1, h2, h3 = _split3(y2t)
    xs1, xs2, xs3 = _split3(xsq)
    ys1, ys2, ys3 = _split3(ysq)
    ones = np.ones(_N, dtype=_BF16)

    xrows, yrows = [], []

    def add(xr, yr):
        xrows.append(xr)
        yrows.append(yr)

    add(ones, ys1)
    add(xs1, ones)
    for d in range(3):
        add(g1[d], h1[d])
    add(ones, ys2)
    add(ones, ys3)
    add(xs2, ones)
    add(xs3, ones)
    for d in range(3):
        add(g2[d], h1[d])
        add(g1[d], h2[d])
        add(g3[d], h1[d])
        add(g2[d], h2[d])
        add(g1[d], h3[d])
        add(g3[d], h2[d])
        add(g2[d], h3[d])
    xh = np.stack(xrows).astype(_BF16)
    # negate the y side so the PE emits -d2 (mins become maxes on-device)
    yh = (-np.stack(yrows).astype(np.float32)).astype(_BF16)
    assert xh.shape == (_K, _N)
    return xh, yh


def kernel(x1, y1):
    from concourse.bass_utils import run_bass_kernel_spmd

    x1 = np.asarray(x1)
    y1 = np.asarray(y1)
    assert x1.shape == (_B, _N, 3) and y1.shape == (_B, _N, 3)

    nc = _get_program()
    in_maps = []
    for b in range(_B):
        xh, yh = _augment(x1[b], y1[b])
        in_maps.append({"xh": xh, "yh": yh})

    res = run_bass_kernel_spmd(nc, in_maps, list(range(_NCORES)))
    total = 0.0
    for c in range(_NCORES):
        total += _host_finish(res.results[c])
    return np.float32(total / (_B * _N))


def _host_finish(r):
    """Assemble one core's outputs into sum(dist_a) + sum(dist_b)."""
    mh = r["outh"].astype(np.float32)       # (128, 15*1024) h3 remnants
    ml = r["outa"].astype(np.float32)       # (128, 16) last pair's max8
    mb = r["outb"].astype(np.float32)       # (128, 4096) dirB accumulator
    # dirA: -d2min per m (m = mt*128 + partition)
    a = np.empty((128, _MT), dtype=np.float32)
    a[:, 0:_MT - 2] = (mh.reshape(128, 15, 2, 512).max(axis=3)
                       .reshape(128, _MT - 2))
    a[:, _MT - 2:_MT] = ml.reshape(128, 2, 8).max(axis=2)
    # dirB: fold in the skipped tiles' raw slabs, then partition max
    b = mb.max(axis=0)
    for i in range(len(_SKIP_TILES)):
        b = np.maximum(b, r[f"outs{i}"].astype(np.float32).max(axis=0))
    dist_a = np.sqrt(1.0e-8 + np.maximum(-a, 0.0), dtype=np.float32)
    dist_b = np.sqrt(1.0e-8 + np.maximum(-b, 0.0), dtype=np.float32)
    return (float(dist_a.sum(dtype=np.float64))
            + float(dist_b.sum(dtype=np.float64)))


# revision 26
# speedup vs baseline: 1.1646x; 1.0390x over previous
"""Chamfer distance on 8 Trainium2 NeuronCores.

Problem: x1 (8, 4096, 3) f32, y1 (8, 4096, 3) f32.
  d2[b,m,n] = |y[b,m] - x[b,n]|^2
  out = mean_{b,n}(min_m sqrt(1e-8 + max(d2,0))) + mean_{b,m}(min_n ...)

Strategy (data-parallel over B, one batch element per core):
  * sqrt / +eps / max(.,0) are monotonic -> compute mins over raw d2 and
    apply them only to the reduced 4096-vectors on the host.
  * -d2 is produced in PSUM by matmuls with augmented K=30 inputs
    (3-level bf16 split of each fp32 operand, ~2^-26 accurate); the y
    side is negated so all on-device mins become maxes (MAX8 usable).
  * the PE runs TWO row-tiled streams (tile rows 0 and 2 of the 32x128
    tiling grid, operands replicated at SBUF partition bases 0 and 64),
    so the two weight/ifmap streams overlap and LDWEIGHTS hides.
  * per m-tile-PAIR the two streams fill one [128, 2, 4096] bf16 slab
    (ScalarE casts the four 2048-col PSUM chunks; this ~1.9us/chunk
    evacuation is pinned to ScalarE to keep the DVE free).
  * the DVE is the bottleneck (~100% busy): it runs, per pair,
      - direction B (min over m per n): 2 running-max tensor_tensors
        into a [128, 4096] accumulator (bf16 2x mode),
      - direction A (min over n per m): a halving max tree BATCHED over
        the pair via 3-D access patterns (halves per level in one op),
        finishing with MAX8 per tile into m8all.
  * outputs: m8all [128, 32*8] bf16 (host takes max of each 8) and the
    dirB accumulator [128, 4096] bf16 (host takes max over partitions).
    Output DMA is split across queues to shorten the tail.
"""

import os
import sys

for _p in ("/opt/trn_rl_repo", "/root/.axon_site/_ro/trn_rl_repo"):
    if os.path.isdir(_p) and _p not in sys.path:
        sys.path.insert(0, _p)
        break

import numpy as np
import ml_dtypes

_B = 8
_N = 4096          # points per cloud (both x and y)
_K = 30            # augmented contraction dim (3-level bf16 split)
_NCORES = 8
_MT = _N // 128    # 32 m-tiles
_CHUNK = 2048      # PSUM chunk (4 banks); 2 chunks per m-tile

_BF16 = ml_dtypes.bfloat16

# knobs
_STREAMS = int(os.environ.get("CH_STREAMS", "2"))   # 1 or 2 PE tile rows
# early tiles excluded from the on-device dirB chain; their bf16 slabs
# ship to DRAM (overlapped DMA) and the host folds them in
_SKIP_TILES = tuple(int(t) for t in
                    os.environ.get("CH_SKIP_TILES", "2,5,8,11").split(",") if t)

_PROGRAM = None


def _build_program():
    import concourse.bacc as bacc
    import concourse.tile as tile
    import concourse.mybir as mybir

    f32 = mybir.dt.float32
    bf16 = mybir.dt.bfloat16
    MAX = mybir.AluOpType.max

    nc = bacc.Bacc("TRN2", target_bir_lowering=False, debug=False,
                   num_devices=_NCORES)

    xh_d = nc.dram_tensor("xh", [_K, _N], bf16, kind="ExternalInput")
    yh_d = nc.dram_tensor("yh", [_K, _N], bf16, kind="ExternalInput")
    # dirA tree shipped at the h2 level (1024 cols per tile) for pairs
    # 0..14; the final pair finishes on-device (avoids a tail DMA)
    outh_d = nc.dram_tensor("outh", [128, 15 * 2048], bf16,
                            kind="ExternalOutput")
    outa_d = nc.dram_tensor("outa", [128, 16], bf16, kind="ExternalOutput")
    outb_d = nc.dram_tensor("outb", [128, _N], bf16, kind="ExternalOutput")
    outs_d = [nc.dram_tensor(f"outs{i}", [128, _N], bf16,
                             kind="ExternalOutput")
              for i in range(len(_SKIP_TILES))]

    with tile.TileContext(nc) as tc:
        with tc.tile_pool(name="singles", bufs=1) as singles:
            # operand replicas at partition bases 0 and 64 (PE tile rows
            # 0 and 2 of the 32x128 row-tiling grid)
            xh_s = singles.tile([128, _N], bf16)
            yh_s = singles.tile([128, _N], bf16)
            m8all = singles.tile([128, 16], bf16)
            accB = singles.tile([128, _N], bf16)

            # input DMAs: sync + gpsimd queues only (the scalar engine is
            # the saturated caster -- its queue must stay DMA-free).
            # Pieces ordered by when the pipeline needs them.
            bases = (0, 64) if _STREAMS == 2 else (0,)
            qmap = {0: nc.sync, 64: nc.gpsimd}
            for base in bases:
                qa = qmap[base]
                qa.dma_start(out=xh_s[base:base + _K, 0:512],
                             in_=xh_d.ap()[:, 0:512])
                qa.dma_start(out=yh_s[base:base + _K, 0:256],
                             in_=yh_d.ap()[:, 0:256])
                qa.dma_start(out=xh_s[base:base + _K, 512:2048],
                             in_=xh_d.ap()[:, 512:2048])
            for base in bases:
                qa = qmap[base]
                qa.dma_start(out=xh_s[base:base + _K, 2048:_N],
                             in_=xh_d.ap()[:, 2048:_N])
            for base in bases:
                qa = qmap[base]
                qa.dma_start(out=yh_s[base:base + _K, 256:2048],
                             in_=yh_d.ap()[:, 256:2048])
                qa.dma_start(out=yh_s[base:base + _K, 2048:_N],
                             in_=yh_d.ap()[:, 2048:_N])

            with tc.tile_pool(name="psum0", bufs=1, space="PSUM") as psum0, \
                 tc.tile_pool(name="psum1", bufs=1, space="PSUM") as psum1, \
                 tc.tile_pool(name="castp", bufs=4) as castp, \
                 tc.tile_pool(name="treep", bufs=3) as treep, \
                 tc.tile_pool(name="h3p", bufs=4) as h3p:
                psum_pools = (psum0, psum1)
                NPAIR = _MT // 2

                def emit_mms(p, c, r, pt):
                    mt = 2 * p + r
                    rb = 64 * r if _STREAMS == 2 else 0
                    lhsT = yh_s[rb:rb + _K, mt * 128:(mt + 1) * 128]
                    for j in range(_CHUNK // 512):
                        n0 = c * _CHUNK + j * 512
                        nc.tensor.matmul(
                            pt[:, j * 512:(j + 1) * 512],
                            lhsT=lhsT,
                            rhs=xh_s[rb:rb + _K, n0:n0 + 512],
                            start=True, stop=True,
                        )

                def acc_update(mt, lo, hi, src):
                    # running dirB max over tiles for columns [lo:hi)
                    if mt == 0:
                        nc.vector.tensor_copy(accB[:, lo:hi], src)
                    else:
                        nc.vector.tensor_tensor(
                            out=accB[:, lo:hi], in0=accB[:, lo:hi],
                            in1=src, op=MAX)

                def lvl1_piece(h1, r, slab, lo, hi):
                    # dirA level-1 fold of slab columns [lo:hi) -> h1 slot
                    w = (hi - lo) // 2
                    nc.vector.tensor_tensor(
                        out=h1[:, r, lo // 2:lo // 2 + w],
                        in0=slab[:, r, lo:lo + w],
                        in1=slab[:, r, lo + w:hi], op=MAX)

                def dira_upper(p, h1):
                    # tree level 2 batched over the pair; the h2 remnant
                    # ships to the host (overlapped DMA) except for the
                    # final pair, which finishes on-device
                    h2 = h3p.tile([128, 2, 1024], bf16, name="h2")
                    nc.vector.tensor_tensor(
                        out=h2[:, :, :], in0=h1[:, :, 0:1024],
                        in1=h1[:, :, 1024:2048], op=MAX)
                    if p == NPAIR - 1:
                        h3 = treep.tile([128, 2, 512], bf16, name="h3")
                        nc.vector.tensor_tensor(
                            out=h3[:, :, :], in0=h2[:, :, 0:512],
                            in1=h2[:, :, 512:1024], op=MAX)
                        h4 = treep.tile([128, 2, 256], bf16, name="h4")
                        nc.vector.tensor_tensor(
                            out=h4[:, :, :], in0=h3[:, :, 0:256],
                            in1=h3[:, :, 256:512], op=MAX)
                        for r in range(2):
                            nc.vector.max(m8all[:, r * 8:(r + 1) * 8],
                                          h4[:, r, :])
                    else:
                        q = nc.sync if p % 2 == 0 else nc.gpsimd
                        q.dma_start(
                            out=outh_d.ap()[:, p * 2048:(p + 1) * 2048],
                            in_=h2[:, :, :])

                # ---- ramp: pairs 0-1 run chunk/piece-granular in a
                # c0-wave-then-c1-wave order matching input-DMA arrival,
                # so the DVE starts as soon as the first pieces are cast
                RAMP = 2
                rslabs = [castp.tile([128, 2, _N], bf16, name="slab")
                          for i in range(RAMP)]
                rh1s = [treep.tile([128, 2, _CHUNK], bf16, name="h1")
                        for i in range(RAMP)]
                for c in range(2):
                    for p in range(RAMP):
                        slab, h1 = rslabs[p], rh1s[p]
                        for r in range(2):
                            mt = 2 * p + r
                            pt = psum_pools[r].tile([128, _CHUNK], f32,
                                                    name=f"pt{r}")
                            emit_mms(p, c, r, pt)
                            base = c * _CHUNK
                            if p == 0 and c == 0:
                                # 1024-col cast pieces so the DVE starts
                                # right after the first one lands
                                nc.scalar.copy(
                                    out=slab[:, r, 0:1024],
                                    in_=pt[:, 0:1024])
                                nc.scalar.copy(
                                    out=slab[:, r, 1024:2048],
                                    in_=pt[:, 1024:2048])
                                for lo in (0, 1024):
                                    if mt not in _SKIP_TILES:
                                        acc_update(mt, lo, lo + 1024,
                                                   slab[:, r, lo:lo + 1024])
                                    lvl1_piece(h1, r, slab, lo, lo + 1024)
                            else:
                                nc.scalar.copy(
                                    out=slab[:, r, base:base + _CHUNK],
                                    in_=pt[:, :])
                                if mt not in _SKIP_TILES:
                                    acc_update(mt, base, base + _CHUNK,
                                               slab[:, r,
                                                    base:base + _CHUNK])
                                lvl1_piece(h1, r, slab, base, base + _CHUNK)
                skip_idx = {mt: i for i, mt in enumerate(_SKIP_TILES)}
                for p in range(RAMP):
                    for r in range(2):
                        mt = 2 * p + r
                        if mt in _SKIP_TILES:
                            q = nc.sync if skip_idx[mt] % 2 else nc.gpsimd
                            q.dma_start(out=outs_d[skip_idx[mt]].ap(),
                                        in_=rslabs[p][:, r, :])
                for p in range(RAMP):
                    dira_upper(p, rh1s[p])

                # ---- steady state: full-tile granularity ----
                for p in range(RAMP, NPAIR - 1):
                    slab = castp.tile([128, 2, _N], bf16, name="slab")
                    for c in range(2):
                        for r in range(2):
                            mt = 2 * p + r
                            pt = psum_pools[r].tile([128, _CHUNK], f32,
                                                    name=f"pt{r}")
                            emit_mms(p, c, r, pt)
                            nc.scalar.copy(
                                out=slab[:, r, c * _CHUNK:(c + 1) * _CHUNK],
                                in_=pt[:, :])
                    for r in range(2):
                        mt = 2 * p + r
                        if mt in _SKIP_TILES:
                            q = nc.sync if skip_idx[mt] % 2 else nc.gpsimd
                            q.dma_start(out=outs_d[skip_idx[mt]].ap(),
                                        in_=slab[:, r, :])
                        else:
                            acc_update(mt, 0, _N, slab[:, r, :])
                    h1 = treep.tile([128, 2, _CHUNK], bf16, name="h1")
                    nc.vector.tensor_tensor(
                        out=h1[:, :, :], in0=slab[:, :, 0:2048],
                        in1=slab[:, :, 2048:4096], op=MAX)
                    dira_upper(p, h1)

                # ---- tail pair: chunk-granular so outb chunk 0 can ship
                # while chunk 1 is still reducing ----
                p = NPAIR - 1
                slab = castp.tile([128, 2, _N], bf16, name="slab")
                h1 = treep.tile([128, 2, _CHUNK], bf16, name="h1")
                for c in range(2):
                    for r in range(2):
                        mt = 2 * p + r
                        pt = psum_pools[r].tile([128, _CHUNK], f32,
                                                name=f"pt{r}")
                        emit_mms(p, c, r, pt)
                        nc.scalar.copy(
                            out=slab[:, r, c * _CHUNK:(c + 1) * _CHUNK],
                            in_=pt[:, :])
                    base = c * _CHUNK
                    for r in range(2):
                        acc_update(2 * p + r, base, base + _CHUNK,
                                   slab[:, r, base:base + _CHUNK])
                        lvl1_piece(h1, r, slab, base, base + _CHUNK)
                    if c == 0:
                        nc.sync.dma_start(out=outb_d.ap()[:, 0:1024],
                                          in_=accB[:, 0:1024])
                        nc.gpsimd.dma_start(out=outb_d.ap()[:, 1024:2048],
                                            in_=accB[:, 1024:2048])
                dira_upper(p, h1)

            # ---- epilogue DMA (split across queues to hide the tail) ----
            nc.gpsimd.dma_start(out=outb_d.ap()[:, 2048:3072],
                                in_=accB[:, 2048:3072])
            nc.sync.dma_start(out=outb_d.ap()[:, 3072:_N],
                              in_=accB[:, 3072:_N])
            nc.sync.dma_start(out=outa_d.ap(), in_=m8all[:, :])

    nc.compile()
    return nc


def _get_program():
    global _PROGRAM
    if _PROGRAM is None:
        _PROGRAM = _build_program()
    return _PROGRAM


def _split3(a):
    """fp32 array -> 3-level bf16 split (h1 + h2 + h3 ~ a to ~2^-26 rel)."""
    h1 = a.astype(_BF16)
    r1 = a - h1.astype(np.float32)
    h2 = r1.astype(_BF16)
    r2 = r1 - h2.astype(np.float32)
    h3 = r2.astype(_BF16)
    return h1, h2, h3


def _augment(x, y):
    """x, y: (4096, 3) f32 -> xh, yh (30, 4096) bf16 such that
    sum_k yh[k, m] * xh[k, n] == -|y[m] - x[n]|^2 to ~1e-6 abs.

    Every fp32 operand is split into 3 bf16 levels; all product pairs down
    to the 2^-24 level are kept, so each product is exact in the PE's fp32
    PSUM accumulation.  Large-magnitude rows (y_sq, x_sq, hi*hi cross
    terms) come first so the running PSUM partial cancels down to ~d2
    early, keeping sequential-accumulation rounding at the fp32 noise
    floor of the reference itself."""
    xt = np.ascontiguousarray(x.T.astype(np.float32))            # (3, N)
    y2t = np.ascontiguousarray((-2.0 * y).T.astype(np.float32))  # (3, N)
    xsq = np.einsum("nd,nd->n", x, x).astype(np.float32)         # (N,)
    ysq = np.einsum("nd,nd->n", y, y).astype(np.float32)

    g1, g2, g3 = _split3(xt)
    h1, h2, h3 = _split3(y2t)
    xs1, xs2, xs3 = _split3(xsq)
    ys1, ys2, ys3 = _split3(ysq)
    ones = np.ones(_N, dtype=_BF16)

    xrows, yrows = [], []

    def add(xr, yr):
        xrows.append(xr)
        yrows.append(yr)

    add(ones, ys1)
    add(xs1, ones)
    for d in range(3):
        add(g1[d], h1[d])
    add(ones, ys2)
    add(ones, ys3)
    add(xs2, ones)
    add(xs3, ones)
    for d in range(3):
        add(g2[d], h1[d])
        add(g1[d], h2[d])
        add(g3[d], h1[d])
        add(g2[d], h2[d])
        add(g1[d], h3[d])
        add(g3[d], h2[d])
        add(g2[d], h3[d])
    xh = np.stack(xrows).astype(_BF16)
    # negate the y side so the PE emits -d2 (mins become maxes on-device)
    yh = (-np.stack(yrows).astype(np.float32)).astype(_BF16)
    assert xh.shape == (_K, _N)
    return xh, yh


def kernel(x1, y1):
    from concourse.bass_utils import run_bass_kernel_spmd

    x1 = np.asarray(x1)
    y1 = np.asarray(y1)
    assert x1.shape == (_B, _N, 3) and y1.shape == (_B, _N, 3)

    nc = _get_program()
    in_maps = []
    for b in range(_B):
        xh, yh = _augment(x1[b], y1[b])
        in_maps.append({"xh": xh, "yh": yh})

    res = run_bass_kernel_spmd(nc, in_maps, list(range(_NCORES)))
    total = 0.0
    for c in range(_NCORES):
        total += _host_finish(res.results[c])
    return np.float32(total / (_B * _N))


def _host_finish(r):
    """Assemble one core's outputs into sum(dist_a) + sum(dist_b)."""
    mh = r["outh"].astype(np.float32)       # (128, 15*2048) h2 remnants
    ml = r["outa"].astype(np.float32)       # (128, 16) last pair's max8
    mb = r["outb"].astype(np.float32)       # (128, 4096) dirB accumulator
    # dirA: -d2min per m (m = mt*128 + partition)
    a = np.empty((128, _MT), dtype=np.float32)
    a[:, 0:_MT - 2] = (mh.reshape(128, 15, 2, 1024).max(axis=3)
                       .reshape(128, _MT - 2))
    a[:, _MT - 2:_MT] = ml.reshape(128, 2, 8).max(axis=2)
    # dirB: fold in the skipped tiles' raw slabs, then partition max
    b = mb.max(axis=0)
    for i in range(len(_SKIP_TILES)):
        b = np.maximum(b, r[f"outs{i}"].astype(np.float32).max(axis=0))
    dist_a = np.sqrt(1.0e-8 + np.maximum(-a, 0.0), dtype=np.float32)
    dist_b = np.sqrt(1.0e-8 + np.maximum(-b, 0.0), dtype=np.float32)
    return (float(dist_a.sum(dtype=np.float64))
            + float(dist_b.sum(dtype=np.float64)))


# revision 27
# speedup vs baseline: 1.1799x; 1.0131x over previous
"""Chamfer distance on 8 Trainium2 NeuronCores.

Problem: x1 (8, 4096, 3) f32, y1 (8, 4096, 3) f32.
  d2[b,m,n] = |y[b,m] - x[b,n]|^2
  out = mean_{b,n}(min_m sqrt(1e-8 + max(d2,0))) + mean_{b,m}(min_n ...)

Strategy (data-parallel over B, one batch element per core):
  * sqrt / +eps / max(.,0) are monotonic -> compute mins over raw d2 and
    apply them only to the reduced 4096-vectors on the host.
  * -d2 is produced in PSUM by matmuls with augmented K=30 inputs
    (3-level bf16 split of each fp32 operand, ~2^-26 accurate); the y
    side is negated so all on-device mins become maxes (MAX8 usable).
  * the PE runs TWO row-tiled streams (tile rows 0 and 2 of the 32x128
    tiling grid, operands replicated at SBUF partition bases 0 and 64),
    so the two weight/ifmap streams overlap and LDWEIGHTS hides.
  * per m-tile-PAIR the two streams fill one [128, 2, 4096] bf16 slab
    (ScalarE casts the four 2048-col PSUM chunks; this ~1.9us/chunk
    evacuation is pinned to ScalarE to keep the DVE free).
  * the DVE is the bottleneck (~100% busy): it runs, per pair,
      - direction B (min over m per n): 2 running-max tensor_tensors
        into a [128, 4096] accumulator (bf16 2x mode),
      - direction A (min over n per m): a halving max tree BATCHED over
        the pair via 3-D access patterns (halves per level in one op),
        finishing with MAX8 per tile into m8all.
  * outputs: m8all [128, 32*8] bf16 (host takes max of each 8) and the
    dirB accumulator [128, 4096] bf16 (host takes max over partitions).
    Output DMA is split across queues to shorten the tail.
"""

import os
import sys

for _p in ("/opt/trn_rl_repo", "/root/.axon_site/_ro/trn_rl_repo"):
    if os.path.isdir(_p) and _p not in sys.path:
        sys.path.insert(0, _p)
        break

import numpy as np
import ml_dtypes

_B = 8
_N = 4096          # points per cloud (both x and y)
_K = 30            # augmented contraction dim (3-level bf16 split)
_NCORES = 8
_MT = _N // 128    # 32 m-tiles
_CHUNK = 2048      # PSUM chunk (4 banks); 2 chunks per m-tile

_BF16 = ml_dtypes.bfloat16

# knobs
_STREAMS = int(os.environ.get("CH_STREAMS", "2"))   # 1 or 2 PE tile rows
# early tiles excluded from the on-device dirB chain; their bf16 slabs
# ship to DRAM (overlapped DMA) and the host folds them in
_SKIP_TILES = tuple(int(t) for t in
                    os.environ.get("CH_SKIP_TILES", "2,5,8,11,26,29").split(",") if t)

_PROGRAM = None


def _build_program():
    import concourse.bacc as bacc
    import concourse.tile as tile
    import concourse.mybir as mybir

    f32 = mybir.dt.float32
    bf16 = mybir.dt.bfloat16
    MAX = mybir.AluOpType.max

    nc = bacc.Bacc("TRN2", target_bir_lowering=False, debug=False,
                   num_devices=_NCORES)

    xh_d = nc.dram_tensor("xh", [_K, _N], bf16, kind="ExternalInput")
    yh_d = nc.dram_tensor("yh", [_K, _N], bf16, kind="ExternalInput")
    # dirA tree shipped at the h2 level (1024 cols per tile)
    outh_d = nc.dram_tensor("outh", [128, 16 * 2048], bf16,
                            kind="ExternalOutput")
    outb_d = nc.dram_tensor("outb", [128, _N], bf16, kind="ExternalOutput")
    outs_d = [nc.dram_tensor(f"outs{i}", [128, _N], bf16,
                             kind="ExternalOutput")
              for i in range(len(_SKIP_TILES))]

    with tile.TileContext(nc) as tc:
        with tc.tile_pool(name="singles", bufs=1) as singles:
            # operand replicas at partition bases 0 and 64 (PE tile rows
            # 0 and 2 of the 32x128 row-tiling grid)
            xh_s = singles.tile([128, _N], bf16)
            yh_s = singles.tile([128, _N], bf16)
            accB = singles.tile([128, _N], bf16)

            # input DMAs: sync + gpsimd queues only (the scalar engine is
            # the saturated caster -- its queue must stay DMA-free).
            # Pieces ordered by when the pipeline needs them.
            bases = (0, 64) if _STREAMS == 2 else (0,)
            qmap = {0: nc.sync, 64: nc.gpsimd}
            for base in bases:
                qa = qmap[base]
                qa.dma_start(out=xh_s[base:base + _K, 0:512],
                             in_=xh_d.ap()[:, 0:512])
                qa.dma_start(out=yh_s[base:base + _K, 0:256],
                             in_=yh_d.ap()[:, 0:256])
                qa.dma_start(out=xh_s[base:base + _K, 512:2048],
                             in_=xh_d.ap()[:, 512:2048])
            for base in bases:
                qa = qmap[base]
                qa.dma_start(out=xh_s[base:base + _K, 2048:_N],
                             in_=xh_d.ap()[:, 2048:_N])
            for base in bases:
                qa = qmap[base]
                qa.dma_start(out=yh_s[base:base + _K, 256:2048],
                             in_=yh_d.ap()[:, 256:2048])
                qa.dma_start(out=yh_s[base:base + _K, 2048:_N],
                             in_=yh_d.ap()[:, 2048:_N])

            with tc.tile_pool(name="psum0", bufs=1, space="PSUM") as psum0, \
                 tc.tile_pool(name="psum1", bufs=1, space="PSUM") as psum1, \
                 tc.tile_pool(name="castp", bufs=4) as castp, \
                 tc.tile_pool(name="treep", bufs=3) as treep, \
                 tc.tile_pool(name="h3p", bufs=4) as h3p:
                psum_pools = (psum0, psum1)
                NPAIR = _MT // 2

                def emit_mms(p, c, r, pt):
                    mt = 2 * p + r
                    rb = 64 * r if _STREAMS == 2 else 0
                    lhsT = yh_s[rb:rb + _K, mt * 128:(mt + 1) * 128]
                    for j in range(_CHUNK // 512):
                        n0 = c * _CHUNK + j * 512
                        nc.tensor.matmul(
                            pt[:, j * 512:(j + 1) * 512],
                            lhsT=lhsT,
                            rhs=xh_s[rb:rb + _K, n0:n0 + 512],
                            start=True, stop=True,
                        )

                def acc_update(mt, lo, hi, src):
                    # running dirB max over tiles for columns [lo:hi)
                    if mt == 0:
                        nc.vector.tensor_copy(accB[:, lo:hi], src)
                    else:
                        nc.vector.tensor_tensor(
                            out=accB[:, lo:hi], in0=accB[:, lo:hi],
                            in1=src, op=MAX)

                def lvl1_piece(h1, r, slab, lo, hi):
                    # dirA level-1 fold of slab columns [lo:hi) -> h1 slot
                    w = (hi - lo) // 2
                    nc.vector.tensor_tensor(
                        out=h1[:, r, lo // 2:lo // 2 + w],
                        in0=slab[:, r, lo:lo + w],
                        in1=slab[:, r, lo + w:hi], op=MAX)

                def dira_upper(p, h1):
                    # tree level 2 batched over the pair; the h2 remnant
                    # ships to the host via overlapped DMA
                    h2 = h3p.tile([128, 2, 1024], bf16, name="h2")
                    nc.vector.tensor_tensor(
                        out=h2[:, :, :], in0=h1[:, :, 0:1024],
                        in1=h1[:, :, 1024:2048], op=MAX)
                    q = nc.sync if p % 2 == 0 else nc.gpsimd
                    q.dma_start(
                        out=outh_d.ap()[:, p * 2048:(p + 1) * 2048],
                        in_=h2[:, :, :])

                # ---- ramp: pairs 0-1 run chunk/piece-granular in a
                # c0-wave-then-c1-wave order matching input-DMA arrival,
                # so the DVE starts as soon as the first pieces are cast
                RAMP = 2
                rslabs = [castp.tile([128, 2, _N], bf16, name="slab")
                          for i in range(RAMP)]
                rh1s = [treep.tile([128, 2, _CHUNK], bf16, name="h1")
                        for i in range(RAMP)]
                for c in range(2):
                    for p in range(RAMP):
                        slab, h1 = rslabs[p], rh1s[p]
                        for r in range(2):
                            mt = 2 * p + r
                            pt = psum_pools[r].tile([128, _CHUNK], f32,
                                                    name=f"pt{r}")
                            emit_mms(p, c, r, pt)
                            base = c * _CHUNK
                            if p == 0 and c == 0:
                                # 1024-col cast pieces so the DVE starts
                                # right after the first one lands
                                nc.scalar.copy(
                                    out=slab[:, r, 0:1024],
                                    in_=pt[:, 0:1024])
                                nc.scalar.copy(
                                    out=slab[:, r, 1024:2048],
                                    in_=pt[:, 1024:2048])
                                for lo in (0, 1024):
                                    if mt not in _SKIP_TILES:
                                        acc_update(mt, lo, lo + 1024,
                                                   slab[:, r, lo:lo + 1024])
                                    lvl1_piece(h1, r, slab, lo, lo + 1024)
                            else:
                                nc.scalar.copy(
                                    out=slab[:, r, base:base + _CHUNK],
                                    in_=pt[:, :])
                                if mt not in _SKIP_TILES:
                                    acc_update(mt, base, base + _CHUNK,
                                               slab[:, r,
                                                    base:base + _CHUNK])
                                lvl1_piece(h1, r, slab, base, base + _CHUNK)
                skip_idx = {mt: i for i, mt in enumerate(_SKIP_TILES)}
                for p in range(RAMP):
                    for r in range(2):
                        mt = 2 * p + r
                        if mt in _SKIP_TILES:
                            q = nc.sync if skip_idx[mt] % 2 else nc.gpsimd
                            q.dma_start(out=outs_d[skip_idx[mt]].ap(),
                                        in_=rslabs[p][:, r, :])
                for p in range(RAMP):
                    dira_upper(p, rh1s[p])

                # ---- steady state: full-tile granularity ----
                for p in range(RAMP, NPAIR - 1):
                    slab = castp.tile([128, 2, _N], bf16, name="slab")
                    for c in range(2):
                        for r in range(2):
                            mt = 2 * p + r
                            pt = psum_pools[r].tile([128, _CHUNK], f32,
                                                    name=f"pt{r}")
                            emit_mms(p, c, r, pt)
                            nc.scalar.copy(
                                out=slab[:, r, c * _CHUNK:(c + 1) * _CHUNK],
                                in_=pt[:, :])
                    for r in range(2):
                        mt = 2 * p + r
                        if mt in _SKIP_TILES:
                            q = nc.sync if skip_idx[mt] % 2 else nc.gpsimd
                            q.dma_start(out=outs_d[skip_idx[mt]].ap(),
                                        in_=slab[:, r, :])
                        else:
                            acc_update(mt, 0, _N, slab[:, r, :])
                    h1 = treep.tile([128, 2, _CHUNK], bf16, name="h1")
                    nc.vector.tensor_tensor(
                        out=h1[:, :, :], in0=slab[:, :, 0:2048],
                        in1=slab[:, :, 2048:4096], op=MAX)
                    dira_upper(p, h1)

                # ---- tail pair: chunk-granular so outb chunk 0 can ship
                # while chunk 1 is still reducing ----
                p = NPAIR - 1
                slab = castp.tile([128, 2, _N], bf16, name="slab")
                h1 = treep.tile([128, 2, _CHUNK], bf16, name="h1")
                for c in range(2):
                    for r in range(2):
                        mt = 2 * p + r
                        pt = psum_pools[r].tile([128, _CHUNK], f32,
                                                name=f"pt{r}")
                        emit_mms(p, c, r, pt)
                        nc.scalar.copy(
                            out=slab[:, r, c * _CHUNK:(c + 1) * _CHUNK],
                            in_=pt[:, :])
                    base = c * _CHUNK
                    for r in range(2):
                        acc_update(2 * p + r, base, base + _CHUNK,
                                   slab[:, r, base:base + _CHUNK])
                        lvl1_piece(h1, r, slab, base, base + _CHUNK)
                    if c == 0:
                        nc.sync.dma_start(out=outb_d.ap()[:, 0:1024],
                                          in_=accB[:, 0:1024])
                        nc.gpsimd.dma_start(out=outb_d.ap()[:, 1024:2048],
                                            in_=accB[:, 1024:2048])
                dira_upper(p, h1)

            # ---- epilogue DMA (split across queues to hide the tail) ----
            nc.gpsimd.dma_start(out=outb_d.ap()[:, 2048:3072],
                                in_=accB[:, 2048:3072])
            nc.sync.dma_start(out=outb_d.ap()[:, 3072:_N],
                              in_=accB[:, 3072:_N])


    nc.compile()
    return nc


def _get_program():
    global _PROGRAM
    if _PROGRAM is None:
        _PROGRAM = _build_program()
    return _PROGRAM


def _split3(a):
    """fp32 array -> 3-level bf16 split (h1 + h2 + h3 ~ a to ~2^-26 rel)."""
    h1 = a.astype(_BF16)
    r1 = a - h1.astype(np.float32)
    h2 = r1.astype(_BF16)
    r2 = r1 - h2.astype(np.float32)
    h3 = r2.astype(_BF16)
    return h1, h2, h3


def _augment(x, y):
    """x, y: (4096, 3) f32 -> xh, yh (30, 4096) bf16 such that
    sum_k yh[k, m] * xh[k, n] == -|y[m] - x[n]|^2 to ~1e-6 abs.

    Every fp32 operand is split into 3 bf16 levels; all product pairs down
    to the 2^-24 level are kept, so each product is exact in the PE's fp32
    PSUM accumulation.  Large-magnitude rows (y_sq, x_sq, hi*hi cross
    terms) come first so the running PSUM partial cancels down to ~d2
    early, keeping sequential-accumulation rounding at the fp32 noise
    floor of the reference itself."""
    xt = np.ascontiguousarray(x.T.astype(np.float32))            # (3, N)
    y2t = np.ascontiguousarray((-2.0 * y).T.astype(np.float32))  # (3, N)
    xsq = np.einsum("nd,nd->n", x, x).astype(np.float32)         # (N,)
    ysq = np.einsum("nd,nd->n", y, y).astype(np.float32)

    g1, g2, g3 = _split3(xt)
    h1, h2, h3 = _split3(y2t)
    xs1, xs2, xs3 = _split3(xsq)
    ys1, ys2, ys3 = _split3(ysq)
    ones = np.ones(_N, dtype=_BF16)

    xrows, yrows = [], []

    def add(xr, yr):
        xrows.append(xr)
        yrows.append(yr)

    add(ones, ys1)
    add(xs1, ones)
    for d in range(3):
        add(g1[d], h1[d])
    add(ones, ys2)
    add(ones, ys3)
    add(xs2, ones)
    add(xs3, ones)
    for d in range(3):
        add(g2[d], h1[d])
        add(g1[d], h2[d])
        add(g3[d], h1[d])
        add(g2[d], h2[d])
        add(g1[d], h3[d])
        add(g3[d], h2[d])
        add(g2[d], h3[d])
    xh = np.stack(xrows).astype(_BF16)
    # negate the y side so the PE emits -d2 (mins become maxes on-device)
    yh = (-np.stack(yrows).astype(np.float32)).astype(_BF16)
    assert xh.shape == (_K, _N)
    return xh, yh


def kernel(x1, y1):
    from concourse.bass_utils import run_bass_kernel_spmd

    x1 = np.asarray(x1)
    y1 = np.asarray(y1)
    assert x1.shape == (_B, _N, 3) and y1.shape == (_B, _N, 3)

    nc = _get_program()
    in_maps = []
    for b in range(_B):
        xh, yh = _augment(x1[b], y1[b])
        in_maps.append({"xh": xh, "yh": yh})

    res = run_bass_kernel_spmd(nc, in_maps, list(range(_NCORES)))
    total = 0.0
    for c in range(_NCORES):
        total += _host_finish(res.results[c])
    return np.float32(total / (_B * _N))


def _host_finish(r):
    """Assemble one core's outputs into sum(dist_a) + sum(dist_b)."""
    mh = r["outh"].astype(np.float32)       # (128, 16*2048) h2 remnants
    mb = r["outb"].astype(np.float32)       # (128, 4096) dirB accumulator
    # dirA: -d2min per m (m = mt*128 + partition)
    a = mh.reshape(128, 16, 2, 1024).max(axis=3).reshape(128, _MT)
    # dirB: fold in the skipped tiles' raw slabs, then partition max
    b = mb.max(axis=0)
    for i in range(len(_SKIP_TILES)):
        b = np.maximum(b, r[f"outs{i}"].astype(np.float32).max(axis=0))
    dist_a = np.sqrt(1.0e-8 + np.maximum(-a, 0.0), dtype=np.float32)
    dist_b = np.sqrt(1.0e-8 + np.maximum(-b, 0.0), dtype=np.float32)
    return (float(dist_a.sum(dtype=np.float64))
            + float(dist_b.sum(dtype=np.float64)))


# revision 28
# speedup vs baseline: 1.1812x; 1.0011x over previous
"""Chamfer distance on 8 Trainium2 NeuronCores.

Problem: x1 (8, 4096, 3) f32, y1 (8, 4096, 3) f32.
  d2[b,m,n] = |y[b,m] - x[b,n]|^2
  out = mean_{b,n}(min_m sqrt(1e-8 + max(d2,0))) + mean_{b,m}(min_n ...)

Strategy (data-parallel over B, one batch element per core):
  * sqrt / +eps / max(.,0) are monotonic -> compute mins over raw d2 and
    apply them only to the reduced 4096-vectors on the host.
  * -d2 is produced in PSUM by matmuls with augmented K=30 inputs
    (3-level bf16 split of each fp32 operand, ~2^-26 accurate); the y
    side is negated so all on-device mins become maxes (MAX8 usable).
  * the PE runs TWO row-tiled streams (tile rows 0 and 2 of the 32x128
    tiling grid, operands replicated at SBUF partition bases 0 and 64),
    so the two weight/ifmap streams overlap and LDWEIGHTS hides.
  * per m-tile-PAIR the two streams fill one [128, 2, 4096] bf16 slab
    (ScalarE casts the four 2048-col PSUM chunks; this ~1.9us/chunk
    evacuation is pinned to ScalarE to keep the DVE free).
  * the DVE is the bottleneck (~100% busy): it runs, per pair,
      - direction B (min over m per n): 2 running-max tensor_tensors
        into a [128, 4096] accumulator (bf16 2x mode),
      - direction A (min over n per m): a halving max tree BATCHED over
        the pair via 3-D access patterns (halves per level in one op),
        finishing with MAX8 per tile into m8all.
  * outputs: m8all [128, 32*8] bf16 (host takes max of each 8) and the
    dirB accumulator [128, 4096] bf16 (host takes max over partitions).
    Output DMA is split across queues to shorten the tail.
"""

import os
import sys

for _p in ("/opt/trn_rl_repo", "/root/.axon_site/_ro/trn_rl_repo"):
    if os.path.isdir(_p) and _p not in sys.path:
        sys.path.insert(0, _p)
        break

import numpy as np
import ml_dtypes

_B = 8
_N = 4096          # points per cloud (both x and y)
_K = 30            # augmented contraction dim (3-level bf16 split)
_NCORES = 8
_MT = _N // 128    # 32 m-tiles
_CHUNK = 2048      # PSUM chunk (4 banks); 2 chunks per m-tile

_BF16 = ml_dtypes.bfloat16

# knobs
_STREAMS = int(os.environ.get("CH_STREAMS", "2"))   # 1 or 2 PE tile rows
# early tiles excluded from the on-device dirB chain; their bf16 slabs
# ship to DRAM (overlapped DMA) and the host folds them in
_SKIP_TILES = tuple(int(t) for t in
                    os.environ.get("CH_SKIP_TILES", "2,5,8,11,17,20,26,29").split(",") if t)

_PROGRAM = None


def _build_program():
    import concourse.bacc as bacc
    import concourse.tile as tile
    import concourse.mybir as mybir

    f32 = mybir.dt.float32
    bf16 = mybir.dt.bfloat16
    MAX = mybir.AluOpType.max

    nc = bacc.Bacc("TRN2", target_bir_lowering=False, debug=False,
                   num_devices=_NCORES)

    xh_d = nc.dram_tensor("xh", [_K, _N], bf16, kind="ExternalInput")
    yh_d = nc.dram_tensor("yh", [_K, _N], bf16, kind="ExternalInput")
    # dirA tree shipped at the h2 level (1024 cols per tile)
    outh_d = nc.dram_tensor("outh", [128, 16 * 2048], bf16,
                            kind="ExternalOutput")
    outb_d = nc.dram_tensor("outb", [128, _N], bf16, kind="ExternalOutput")
    outs_d = [nc.dram_tensor(f"outs{i}", [128, _N], bf16,
                             kind="ExternalOutput")
              for i in range(len(_SKIP_TILES))]

    with tile.TileContext(nc) as tc:
        with tc.tile_pool(name="singles", bufs=1) as singles:
            # operand replicas at partition bases 0 and 64 (PE tile rows
            # 0 and 2 of the 32x128 row-tiling grid)
            xh_s = singles.tile([128, _N], bf16)
            yh_s = singles.tile([128, _N], bf16)
            accB = singles.tile([128, _N], bf16)

            # input DMAs: sync + gpsimd queues only (the scalar engine is
            # the saturated caster -- its queue must stay DMA-free).
            # Pieces ordered by when the pipeline needs them.
            bases = (0, 64) if _STREAMS == 2 else (0,)
            qmap = {0: nc.sync, 64: nc.gpsimd}
            for base in bases:
                qa = qmap[base]
                qa.dma_start(out=xh_s[base:base + _K, 0:512],
                             in_=xh_d.ap()[:, 0:512])
                qa.dma_start(out=yh_s[base:base + _K, 0:256],
                             in_=yh_d.ap()[:, 0:256])
                qa.dma_start(out=xh_s[base:base + _K, 512:2048],
                             in_=xh_d.ap()[:, 512:2048])
            for base in bases:
                qa = qmap[base]
                qa.dma_start(out=xh_s[base:base + _K, 2048:_N],
                             in_=xh_d.ap()[:, 2048:_N])
            for base in bases:
                qa = qmap[base]
                qa.dma_start(out=yh_s[base:base + _K, 256:2048],
                             in_=yh_d.ap()[:, 256:2048])
                qa.dma_start(out=yh_s[base:base + _K, 2048:_N],
                             in_=yh_d.ap()[:, 2048:_N])

            with tc.tile_pool(name="psum0", bufs=1, space="PSUM") as psum0, \
                 tc.tile_pool(name="psum1", bufs=1, space="PSUM") as psum1, \
                 tc.tile_pool(name="castp", bufs=4) as castp, \
                 tc.tile_pool(name="treep", bufs=3) as treep, \
                 tc.tile_pool(name="h3p", bufs=4) as h3p:
                psum_pools = (psum0, psum1)
                NPAIR = _MT // 2

                def emit_mms(p, c, r, pt):
                    mt = 2 * p + r
                    rb = 64 * r if _STREAMS == 2 else 0
                    lhsT = yh_s[rb:rb + _K, mt * 128:(mt + 1) * 128]
                    for j in range(_CHUNK // 512):
                        n0 = c * _CHUNK + j * 512
                        nc.tensor.matmul(
                            pt[:, j * 512:(j + 1) * 512],
                            lhsT=lhsT,
                            rhs=xh_s[rb:rb + _K, n0:n0 + 512],
                            start=True, stop=True,
                        )

                def acc_update(mt, lo, hi, src):
                    # running dirB max over tiles for columns [lo:hi)
                    if mt == 0:
                        nc.vector.tensor_copy(accB[:, lo:hi], src)
                    else:
                        nc.vector.tensor_tensor(
                            out=accB[:, lo:hi], in0=accB[:, lo:hi],
                            in1=src, op=MAX)

                def lvl1_piece(h1, r, slab, lo, hi):
                    # dirA level-1 fold of slab columns [lo:hi) -> h1 slot
                    w = (hi - lo) // 2
                    nc.vector.tensor_tensor(
                        out=h1[:, r, lo // 2:lo // 2 + w],
                        in0=slab[:, r, lo:lo + w],
                        in1=slab[:, r, lo + w:hi], op=MAX)

                def dira_upper(p, h1):
                    # tree level 2 batched over the pair; the h2 remnant
                    # ships to the host via overlapped DMA
                    h2 = h3p.tile([128, 2, 1024], bf16, name="h2")
                    nc.vector.tensor_tensor(
                        out=h2[:, :, :], in0=h1[:, :, 0:1024],
                        in1=h1[:, :, 1024:2048], op=MAX)
                    q = nc.sync if p % 2 == 0 else nc.gpsimd
                    q.dma_start(
                        out=outh_d.ap()[:, p * 2048:(p + 1) * 2048],
                        in_=h2[:, :, :])

                # ---- ramp: pairs 0-1 run chunk/piece-granular in a
                # c0-wave-then-c1-wave order matching input-DMA arrival,
                # so the DVE starts as soon as the first pieces are cast
                RAMP = 2
                rslabs = [castp.tile([128, 2, _N], bf16, name="slab")
                          for i in range(RAMP)]
                rh1s = [treep.tile([128, 2, _CHUNK], bf16, name="h1")
                        for i in range(RAMP)]
                for c in range(2):
                    for p in range(RAMP):
                        slab, h1 = rslabs[p], rh1s[p]
                        for r in range(2):
                            mt = 2 * p + r
                            pt = psum_pools[r].tile([128, _CHUNK], f32,
                                                    name=f"pt{r}")
                            emit_mms(p, c, r, pt)
                            base = c * _CHUNK
                            if p == 0 and c == 0:
                                # 1024-col cast pieces so the DVE starts
                                # right after the first one lands
                                nc.scalar.copy(
                                    out=slab[:, r, 0:1024],
                                    in_=pt[:, 0:1024])
                                nc.scalar.copy(
                                    out=slab[:, r, 1024:2048],
                                    in_=pt[:, 1024:2048])
                                for lo in (0, 1024):
                                    if mt not in _SKIP_TILES:
                                        acc_update(mt, lo, lo + 1024,
                                                   slab[:, r, lo:lo + 1024])
                                    lvl1_piece(h1, r, slab, lo, lo + 1024)
                            else:
                                nc.scalar.copy(
                                    out=slab[:, r, base:base + _CHUNK],
                                    in_=pt[:, :])
                                if mt not in _SKIP_TILES:
                                    acc_update(mt, base, base + _CHUNK,
                                               slab[:, r,
                                                    base:base + _CHUNK])
                                lvl1_piece(h1, r, slab, base, base + _CHUNK)
                skip_idx = {mt: i for i, mt in enumerate(_SKIP_TILES)}
                for p in range(RAMP):
                    for r in range(2):
                        mt = 2 * p + r
                        if mt in _SKIP_TILES:
                            q = nc.sync if skip_idx[mt] % 2 else nc.gpsimd
                            q.dma_start(out=outs_d[skip_idx[mt]].ap(),
                                        in_=rslabs[p][:, r, :])
                for p in range(RAMP):
                    dira_upper(p, rh1s[p])

                # ---- steady state: full-tile granularity ----
                for p in range(RAMP, NPAIR - 1):
                    slab = castp.tile([128, 2, _N], bf16, name="slab")
                    for c in range(2):
                        for r in range(2):
                            mt = 2 * p + r
                            pt = psum_pools[r].tile([128, _CHUNK], f32,
                                                    name=f"pt{r}")
                            emit_mms(p, c, r, pt)
                            nc.scalar.copy(
                                out=slab[:, r, c * _CHUNK:(c + 1) * _CHUNK],
                                in_=pt[:, :])
                    for r in range(2):
                        mt = 2 * p + r
                        if mt in _SKIP_TILES:
                            q = nc.sync if skip_idx[mt] % 2 else nc.gpsimd
                            q.dma_start(out=outs_d[skip_idx[mt]].ap(),
                                        in_=slab[:, r, :])
                        else:
                            acc_update(mt, 0, _N, slab[:, r, :])
                    h1 = treep.tile([128, 2, _CHUNK], bf16, name="h1")
                    nc.vector.tensor_tensor(
                        out=h1[:, :, :], in0=slab[:, :, 0:2048],
                        in1=slab[:, :, 2048:4096], op=MAX)
                    dira_upper(p, h1)

                # ---- tail pair: chunk-granular so outb chunk 0 can ship
                # while chunk 1 is still reducing ----
                p = NPAIR - 1
                slab = castp.tile([128, 2, _N], bf16, name="slab")
                h1 = treep.tile([128, 2, _CHUNK], bf16, name="h1")
                for c in range(2):
                    for r in range(2):
                        mt = 2 * p + r
                        pt = psum_pools[r].tile([128, _CHUNK], f32,
                                                name=f"pt{r}")
                        emit_mms(p, c, r, pt)
                        nc.scalar.copy(
                            out=slab[:, r, c * _CHUNK:(c + 1) * _CHUNK],
                            in_=pt[:, :])
                    base = c * _CHUNK
                    for r in range(2):
                        acc_update(2 * p + r, base, base + _CHUNK,
                                   slab[:, r, base:base + _CHUNK])
                        lvl1_piece(h1, r, slab, base, base + _CHUNK)
                    if c == 0:
                        for i, lo in enumerate(range(0, 2048, 512)):
                            q = nc.sync if i % 2 == 0 else nc.gpsimd
                            q.dma_start(out=outb_d.ap()[:, lo:lo + 512],
                                        in_=accB[:, lo:lo + 512])
                dira_upper(p, h1)

            # ---- epilogue DMA (split across queues to hide the tail) ----
            for i, lo in enumerate(range(2048, _N, 512)):
                q = nc.sync if i % 2 == 0 else nc.gpsimd
                q.dma_start(out=outb_d.ap()[:, lo:lo + 512],
                            in_=accB[:, lo:lo + 512])


    nc.compile()
    return nc


def _get_program():
    global _PROGRAM
    if _PROGRAM is None:
        _PROGRAM = _build_program()
    return _PROGRAM


def _split3(a):
    """fp32 array -> 3-level bf16 split (h1 + h2 + h3 ~ a to ~2^-26 rel)."""
    h1 = a.astype(_BF16)
    r1 = a - h1.astype(np.float32)
    h2 = r1.astype(_BF16)
    r2 = r1 - h2.astype(np.float32)
    h3 = r2.astype(_BF16)
    return h1, h2, h3


def _augment(x, y):
    """x, y: (4096, 3) f32 -> xh, yh (30, 4096) bf16 such that
    sum_k yh[k, m] * xh[k, n] == -|y[m] - x[n]|^2 to ~1e-6 abs.

    Every fp32 operand is split into 3 bf16 levels; all product pairs down
    to the 2^-24 level are kept, so each product is exact in the PE's fp32
    PSUM accumulation.  Large-magnitude rows (y_sq, x_sq, hi*hi cross
    terms) come first so the running PSUM partial cancels down to ~d2
    early, keeping sequential-accumulation rounding at the fp32 noise
    floor of the reference itself."""
    xt = np.ascontiguousarray(x.T.astype(np.float32))            # (3, N)
    y2t = np.ascontiguousarray((-2.0 * y).T.astype(np.float32))  # (3, N)
    xsq = np.einsum("nd,nd->n", x, x).astype(np.float32)         # (N,)
    ysq = np.einsum("nd,nd->n", y, y).astype(np.float32)

    g1, g2, g3 = _split3(xt)
    h1, h2, h3 = _split3(y2t)
    xs1, xs2, xs3 = _split3(xsq)
    ys1, ys2, ys3 = _split3(ysq)
    ones = np.ones(_N, dtype=_BF16)

    xrows, yrows = [], []

    def add(xr, yr):
        xrows.append(xr)
        yrows.append(yr)

    add(ones, ys1)
    add(xs1, ones)
    for d in range(3):
        add(g1[d], h1[d])
    add(ones, ys2)
    add(ones, ys3)
    add(xs2, ones)
    add(xs3, ones)
    for d in range(3):
        add(g2[d], h1[d])
        add(g1[d], h2[d])
        add(g3[d], h1[d])
        add(g2[d], h2[d])
        add(g1[d], h3[d])
        add(g3[d], h2[d])
        add(g2[d], h3[d])
    xh = np.stack(xrows).astype(_BF16)
    # negate the y side so the PE emits -d2 (mins become maxes on-device)
    yh = (-np.stack(yrows).astype(np.float32)).astype(_BF16)
    assert xh.shape == (_K, _N)
    return xh, yh


def kernel(x1, y1):
    from concourse.bass_utils import run_bass_kernel_spmd

    x1 = np.asarray(x1)
    y1 = np.asarray(y1)
    assert x1.shape == (_B, _N, 3) and y1.shape == (_B, _N, 3)

    nc = _get_program()
    in_maps = []
    for b in range(_B):
        xh, yh = _augment(x1[b], y1[b])
        in_maps.append({"xh": xh, "yh": yh})

    res = run_bass_kernel_spmd(nc, in_maps, list(range(_NCORES)))
    total = 0.0
    for c in range(_NCORES):
        total += _host_finish(res.results[c])
    return np.float32(total / (_B * _N))


def _host_finish(r):
    """Assemble one core's outputs into sum(dist_a) + sum(dist_b)."""
    mh = r["outh"].astype(np.float32)       # (128, 16*2048) h2 remnants
    mb = r["outb"].astype(np.float32)       # (128, 4096) dirB accumulator
    # dirA: -d2min per m (m = mt*128 + partition)
    a = mh.reshape(128, 16, 2, 1024).max(axis=3).reshape(128, _MT)
    # dirB: fold in the skipped tiles' raw slabs, then partition max
    b = mb.max(axis=0)
    for i in range(len(_SKIP_TILES)):
        b = np.maximum(b, r[f"outs{i}"].astype(np.float32).max(axis=0))
    dist_a = np.sqrt(1.0e-8 + np.maximum(-a, 0.0), dtype=np.float32)
    dist_b = np.sqrt(1.0e-8 + np.maximum(-b, 0.0), dtype=np.float32)
    return (float(dist_a.sum(dtype=np.float64))
            + float(dist_b.sum(dtype=np.float64)))


# revision 29
# speedup vs baseline: 1.1835x; 1.0020x over previous
"""Chamfer distance on 8 Trainium2 NeuronCores.

Problem: x1 (8, 4096, 3) f32, y1 (8, 4096, 3) f32.
  d2[b,m,n] = |y[b,m] - x[b,n]|^2
  out = mean_{b,n}(min_m sqrt(1e-8 + max(d2,0))) + mean_{b,m}(min_n ...)

Strategy (data-parallel over B, one batch element per core):
  * sqrt / +eps / max(.,0) are monotonic -> compute mins over raw d2 and
    apply them only to the reduced 4096-vectors on the host.
  * -d2 is produced in PSUM by matmuls with augmented K=30 inputs
    (3-level bf16 split of each fp32 operand, ~2^-26 accurate); the y
    side is negated so all on-device mins become maxes (MAX8 usable).
  * the PE runs TWO row-tiled streams (tile rows 0 and 2 of the 32x128
    tiling grid, operands replicated at SBUF partition bases 0 and 64),
    so the two weight/ifmap streams overlap and LDWEIGHTS hides.
  * per m-tile-PAIR the two streams fill one [128, 2, 4096] bf16 slab
    (ScalarE casts the four 2048-col PSUM chunks; this ~1.9us/chunk
    evacuation is pinned to ScalarE to keep the DVE free).
  * the DVE runs, per pair,
      - direction B (min over m per n): 2 running-max tensor_tensors
        into a [128, 4096] accumulator (bf16 2x mode),
      - direction A (min over n per m): halving max-tree levels 1-2,
        BATCHED over the pair via 3-D access patterns.
    The PE, ScalarE and DVE conveyors are then balanced to ~127us each;
    the rest of the reduction is offloaded to the host via DMA that
    overlaps compute:
      - each pair's level-2 remnant (h2, [128, 2, 1024] bf16) ships to
        DRAM as it is produced; the host finishes the per-m max,
      - a stride of m-tiles is excluded from the on-device dirB chain;
        their cast slabs ship raw and the host folds them in.
  * outputs: outh [128, 16*2048] (dirA h2 remnants), outb [128, 4096]
    (dirB accumulator; host takes max over partitions + skipped slabs).
    Output DMA is split finely across queues to shorten the tail.
"""

import os
import sys

for _p in ("/opt/trn_rl_repo", "/root/.axon_site/_ro/trn_rl_repo"):
    if os.path.isdir(_p) and _p not in sys.path:
        sys.path.insert(0, _p)
        break

import numpy as np
import ml_dtypes

_B = 8
_N = 4096          # points per cloud (both x and y)
_K = 30            # augmented contraction dim (3-level bf16 split)
_NCORES = 8
_MT = _N // 128    # 32 m-tiles
_CHUNK = 2048      # PSUM chunk (4 banks); 2 chunks per m-tile

_BF16 = ml_dtypes.bfloat16

# knobs
_STREAMS = int(os.environ.get("CH_STREAMS", "2"))   # 1 or 2 PE tile rows
# early tiles excluded from the on-device dirB chain; their bf16 slabs
# ship to DRAM (overlapped DMA) and the host folds them in
_SKIP_TILES = tuple(int(t) for t in
                    os.environ.get("CH_SKIP_TILES", "2,5,8,11,17,20,26,29").split(",") if t)

_PROGRAM = None


def _build_program():
    import concourse.bacc as bacc
    import concourse.tile as tile
    import concourse.mybir as mybir

    f32 = mybir.dt.float32
    bf16 = mybir.dt.bfloat16
    MAX = mybir.AluOpType.max

    nc = bacc.Bacc("TRN2", target_bir_lowering=False, debug=False,
                   num_devices=_NCORES)

    xh_d = nc.dram_tensor("xh", [_K, _N], bf16, kind="ExternalInput")
    yh_d = nc.dram_tensor("yh", [_K, _N], bf16, kind="ExternalInput")
    # dirA tree shipped at the h2 level (1024 cols per tile)
    outh_d = nc.dram_tensor("outh", [128, 16 * 2048], bf16,
                            kind="ExternalOutput")
    outb_d = nc.dram_tensor("outb", [128, _N], bf16, kind="ExternalOutput")
    outs_d = [nc.dram_tensor(f"outs{i}", [128, _N], bf16,
                             kind="ExternalOutput")
              for i in range(len(_SKIP_TILES))]

    with tile.TileContext(nc) as tc:
        with tc.tile_pool(name="singles", bufs=1) as singles:
            # operand replicas at partition bases 0 and 64 (PE tile rows
            # 0 and 2 of the 32x128 row-tiling grid)
            xh_s = singles.tile([128, _N], bf16)
            yh_s = singles.tile([128, _N], bf16)
            accB = singles.tile([128, _N], bf16)

            # input DMAs: sync + gpsimd queues only (the scalar engine is
            # the saturated caster -- its queue must stay DMA-free).
            # Pieces ordered by when the pipeline needs them.
            bases = (0, 64) if _STREAMS == 2 else (0,)
            qmap = {0: nc.sync, 64: nc.gpsimd}
            for base in bases:
                qa = qmap[base]
                qa.dma_start(out=xh_s[base:base + _K, 0:512],
                             in_=xh_d.ap()[:, 0:512])
                qa.dma_start(out=yh_s[base:base + _K, 0:256],
                             in_=yh_d.ap()[:, 0:256])
                qa.dma_start(out=xh_s[base:base + _K, 512:2048],
                             in_=xh_d.ap()[:, 512:2048])
            for base in bases:
                qa = qmap[base]
                qa.dma_start(out=xh_s[base:base + _K, 2048:_N],
                             in_=xh_d.ap()[:, 2048:_N])
            for base in bases:
                qa = qmap[base]
                qa.dma_start(out=yh_s[base:base + _K, 256:2048],
                             in_=yh_d.ap()[:, 256:2048])
                qa.dma_start(out=yh_s[base:base + _K, 2048:_N],
                             in_=yh_d.ap()[:, 2048:_N])

            with tc.tile_pool(name="psum0", bufs=1, space="PSUM") as psum0, \
                 tc.tile_pool(name="psum1", bufs=1, space="PSUM") as psum1, \
                 tc.tile_pool(name="castp", bufs=4) as castp, \
                 tc.tile_pool(name="treep", bufs=3) as treep, \
                 tc.tile_pool(name="h3p", bufs=4) as h3p:
                psum_pools = (psum0, psum1)
                NPAIR = _MT // 2

                def emit_mms(p, c, r, pt):
                    mt = 2 * p + r
                    rb = 64 * r if _STREAMS == 2 else 0
                    lhsT = yh_s[rb:rb + _K, mt * 128:(mt + 1) * 128]
                    for j in range(_CHUNK // 512):
                        n0 = c * _CHUNK + j * 512
                        nc.tensor.matmul(
                            pt[:, j * 512:(j + 1) * 512],
                            lhsT=lhsT,
                            rhs=xh_s[rb:rb + _K, n0:n0 + 512],
                            start=True, stop=True,
                        )

                def acc_update(mt, lo, hi, src):
                    # running dirB max over tiles for columns [lo:hi)
                    if mt == 0:
                        nc.vector.tensor_copy(accB[:, lo:hi], src)
                    else:
                        nc.vector.tensor_tensor(
                            out=accB[:, lo:hi], in0=accB[:, lo:hi],
                            in1=src, op=MAX)

                def lvl1_piece(h1, r, slab, lo, hi):
                    # dirA level-1 fold of slab columns [lo:hi) -> h1 slot
                    w = (hi - lo) // 2
                    nc.vector.tensor_tensor(
                        out=h1[:, r, lo // 2:lo // 2 + w],
                        in0=slab[:, r, lo:lo + w],
                        in1=slab[:, r, lo + w:hi], op=MAX)

                def dira_upper(p, h1):
                    # tree level 2 batched over the pair; the h2 remnant
                    # ships to the host via overlapped DMA
                    h2 = h3p.tile([128, 2, 1024], bf16, name="h2")
                    nc.vector.tensor_tensor(
                        out=h2[:, :, :], in0=h1[:, :, 0:1024],
                        in1=h1[:, :, 1024:2048], op=MAX)
                    q = nc.sync if p % 2 == 0 else nc.gpsimd
                    q.dma_start(
                        out=outh_d.ap()[:, p * 2048:(p + 1) * 2048],
                        in_=h2[:, :, :])

                # ---- ramp: pairs 0-1 run chunk/piece-granular in a
                # c0-wave-then-c1-wave order matching input-DMA arrival,
                # so the DVE starts as soon as the first pieces are cast
                RAMP = 2
                rslabs = [castp.tile([128, 2, _N], bf16, name="slab")
                          for i in range(RAMP)]
                rh1s = [treep.tile([128, 2, _CHUNK], bf16, name="h1")
                        for i in range(RAMP)]
                for c in range(2):
                    for p in range(RAMP):
                        slab, h1 = rslabs[p], rh1s[p]
                        for r in range(2):
                            mt = 2 * p + r
                            pt = psum_pools[r].tile([128, _CHUNK], f32,
                                                    name=f"pt{r}")
                            emit_mms(p, c, r, pt)
                            base = c * _CHUNK
                            if p == 0 and c == 0:
                                # 1024-col cast pieces so the DVE starts
                                # right after the first one lands
                                nc.scalar.copy(
                                    out=slab[:, r, 0:1024],
                                    in_=pt[:, 0:1024])
                                nc.scalar.copy(
                                    out=slab[:, r, 1024:2048],
                                    in_=pt[:, 1024:2048])
                                for lo in (0, 1024):
                                    if mt not in _SKIP_TILES:
                                        acc_update(mt, lo, lo + 1024,
                                                   slab[:, r, lo:lo + 1024])
                                    lvl1_piece(h1, r, slab, lo, lo + 1024)
                            else:
                                nc.scalar.copy(
                                    out=slab[:, r, base:base + _CHUNK],
                                    in_=pt[:, :])
                                if mt not in _SKIP_TILES:
                                    acc_update(mt, base, base + _CHUNK,
                                               slab[:, r,
                                                    base:base + _CHUNK])
                                lvl1_piece(h1, r, slab, base, base + _CHUNK)
                skip_idx = {mt: i for i, mt in enumerate(_SKIP_TILES)}
                for p in range(RAMP):
                    for r in range(2):
                        mt = 2 * p + r
                        if mt in _SKIP_TILES:
                            q = nc.sync if skip_idx[mt] % 2 else nc.gpsimd
                            q.dma_start(out=outs_d[skip_idx[mt]].ap(),
                                        in_=rslabs[p][:, r, :])
                for p in range(RAMP):
                    dira_upper(p, rh1s[p])

                # ---- steady state: full-tile granularity ----
                for p in range(RAMP, NPAIR - 1):
                    slab = castp.tile([128, 2, _N], bf16, name="slab")
                    for c in range(2):
                        for r in range(2):
                            mt = 2 * p + r
                            pt = psum_pools[r].tile([128, _CHUNK], f32,
                                                    name=f"pt{r}")
                            emit_mms(p, c, r, pt)
                            nc.scalar.copy(
                                out=slab[:, r, c * _CHUNK:(c + 1) * _CHUNK],
                                in_=pt[:, :])
                    for r in range(2):
                        mt = 2 * p + r
                        if mt in _SKIP_TILES:
                            q = nc.sync if skip_idx[mt] % 2 else nc.gpsimd
                            q.dma_start(out=outs_d[skip_idx[mt]].ap(),
                                        in_=slab[:, r, :])
                        else:
                            acc_update(mt, 0, _N, slab[:, r, :])
                    h1 = treep.tile([128, 2, _CHUNK], bf16, name="h1")
                    nc.vector.tensor_tensor(
                        out=h1[:, :, :], in0=slab[:, :, 0:2048],
                        in1=slab[:, :, 2048:4096], op=MAX)
                    dira_upper(p, h1)

                # ---- tail pair: chunk-granular so outb chunk 0 can ship
                # while chunk 1 is still reducing ----
                p = NPAIR - 1
                slab = castp.tile([128, 2, _N], bf16, name="slab")
                h1 = treep.tile([128, 2, _CHUNK], bf16, name="h1")
                for c in range(2):
                    for r in range(2):
                        mt = 2 * p + r
                        pt = psum_pools[r].tile([128, _CHUNK], f32,
                                                name=f"pt{r}")
                        emit_mms(p, c, r, pt)
                        nc.scalar.copy(
                            out=slab[:, r, c * _CHUNK:(c + 1) * _CHUNK],
                            in_=pt[:, :])
                    base = c * _CHUNK
                    for r in range(2):
                        acc_update(2 * p + r, base, base + _CHUNK,
                                   slab[:, r, base:base + _CHUNK])
                        lvl1_piece(h1, r, slab, base, base + _CHUNK)
                    if c == 0:
                        for i, lo in enumerate(range(0, 2048, 512)):
                            q = nc.sync if i % 2 == 0 else nc.gpsimd
                            q.dma_start(out=outb_d.ap()[:, lo:lo + 512],
                                        in_=accB[:, lo:lo + 512])
                dira_upper(p, h1)

            # ---- epilogue DMA (split across queues to hide the tail) ----
            for i, lo in enumerate(range(2048, _N, 512)):
                q = nc.sync if i % 2 == 0 else nc.gpsimd
                q.dma_start(out=outb_d.ap()[:, lo:lo + 512],
                            in_=accB[:, lo:lo + 512])


    nc.compile()
    return nc


def _get_program():
    global _PROGRAM
    if _PROGRAM is None:
        _PROGRAM = _build_program()
    return _PROGRAM


def _split3(a):
    """fp32 array -> 3-level bf16 split (h1 + h2 + h3 ~ a to ~2^-26 rel)."""
    h1 = a.astype(_BF16)
    r1 = a - h1.astype(np.float32)
    h2 = r1.astype(_BF16)
    r2 = r1 - h2.astype(np.float32)
    h3 = r2.astype(_BF16)
    return h1, h2, h3


def _augment(x, y):
    """x, y: (4096, 3) f32 -> xh, yh (30, 4096) bf16 such that
    sum_k yh[k, m] * xh[k, n] == -|y[m] - x[n]|^2 to ~1e-6 abs.

    Every fp32 operand is split into 3 bf16 levels; all product pairs down
    to the 2^-24 level are kept, so each product is exact in the PE's fp32
    PSUM accumulation.  Large-magnitude rows (y_sq, x_sq, hi*hi cross
    terms) come first so the running PSUM partial cancels down to ~d2
    early, keeping sequential-accumulation rounding at the fp32 noise
    floor of the reference itself."""
    xt = np.ascontiguousarray(x.T.astype(np.float32))            # (3, N)
    y2t = np.ascontiguousarray((-2.0 * y).T.astype(np.float32))  # (3, N)
    xsq = np.einsum("nd,nd->n", x, x).astype(np.float32)         # (N,)
    ysq = np.einsum("nd,nd->n", y, y).astype(np.float32)

    g1, g2, g3 = _split3(xt)
    h1, h2, h3 = _split3(y2t)
    xs1, xs2, xs3 = _split3(xsq)
    ys1, ys2, ys3 = _split3(ysq)
    ones = np.ones(_N, dtype=_BF16)

    xrows, yrows = [], []

    def add(xr, yr):
        xrows.append(xr)
        yrows.append(yr)

    add(ones, ys1)
    add(xs1, ones)
    for d in range(3):
        add(g1[d], h1[d])
    add(ones, ys2)
    add(ones, ys3)
    add(xs2, ones)
    add(xs3, ones)
    for d in range(3):
        add(g2[d], h1[d])
        add(g1[d], h2[d])
        add(g3[d], h1[d])
        add(g2[d], h2[d])
        add(g1[d], h3[d])
        add(g3[d], h2[d])
        add(g2[d], h3[d])
    xh = np.stack(xrows).astype(_BF16)
    # negate the y side so the PE emits -d2 (mins become maxes on-device)
    yh = (-np.stack(yrows).astype(np.float32)).astype(_BF16)
    assert xh.shape == (_K, _N)
    return xh, yh


def kernel(x1, y1):
    from concourse.bass_utils import run_bass_kernel_spmd

    x1 = np.asarray(x1)
    y1 = np.asarray(y1)
    assert x1.shape == (_B, _N, 3) and y1.shape == (_B, _N, 3)

    nc = _get_program()
    in_maps = []
    for b in range(_B):
        xh, yh = _augment(x1[b], y1[b])
        in_maps.append({"xh": xh, "yh": yh})

    res = run_bass_kernel_spmd(nc, in_maps, list(range(_NCORES)))
    total = 0.0
    for c in range(_NCORES):
        total += _host_finish(res.results[c])
    return np.float32(total / (_B * _N))


def _host_finish(r):
    """Assemble one core's outputs into sum(dist_a) + sum(dist_b)."""
    mh = r["outh"].astype(np.float32)       # (128, 16*2048) h2 remnants
    mb = r["outb"].astype(np.float32)       # (128, 4096) dirB accumulator
    # dirA: -d2min per m (m = mt*128 + partition)
    a = mh.reshape(128, 16, 2, 1024).max(axis=3).reshape(128, _MT)
    # dirB: fold in the skipped tiles' raw slabs, then partition max
    b = mb.max(axis=0)
    for i in range(len(_SKIP_TILES)):
        b = np.maximum(b, r[f"outs{i}"].astype(np.float32).max(axis=0))
    dist_a = np.sqrt(1.0e-8 + np.maximum(-a, 0.0), dtype=np.float32)
    dist_b = np.sqrt(1.0e-8 + np.maximum(-b, 0.0), dtype=np.float32)
    return (float(dist_a.sum(dtype=np.float64))
            + float(dist_b.sum(dtype=np.float64)))
